# revision 1
# baseline (speedup 1.0000x reference)
"""Trainium2 Bass kernel v3 for NBFNet: single-table HBM dma_gather
(negative-int16 mid-base trick), wavefront truncation, per-layer node
reordering with cross-core equalized classes, column-major DVE trees,
block-diagonal node-phase matmuls."""
import os
import sys
import types
import numpy as np

N = 50000
R = 100
D = 32
L = 6
B = 4
NQ = 4
EPS = 1e-6
NCORES = 8
NPC0 = N // NCORES
NPC = 6272
NTOT = NCORES * NPC
NTBL = NTOT + 16            # + ones token rows
ONES_GPOS = NTOT            # global token position of all-ones token
IDX_BASE = 32768            # gather base offset (mid-table)
ZERO_ENTRY = 2 * R          # rel palette: [0,2R) rels, 2R zero, 2R+1.. bnd
GTILE = 8192
NSEEDN = 4                  # seed region size (nodes)
BUCKETS = np.array([1, 2, 3, 4, 6, 8, 10, 12, 16, 20, 24, 32, 48, 64, 96,
                    128, 192, 256, 384, 512], dtype=np.int64)
STRIPS = [(0, 2176), (2176, 4224), (4224, 6272)]


def _env_setup():
    if "/opt/trn_rl_repo" not in sys.path:
        sys.path.insert(0, "/opt/trn_rl_repo")
    try:
        import antenv  # noqa
        if "antenv.axon_hooks" not in sys.modules:
            hook = [None]
            mod = types.ModuleType("antenv.axon_hooks")
            mod.set_axon_ntff_profile_hook = lambda h: hook.__setitem__(0, h)
            mod.get_axon_ntff_profile_hook = lambda: hook[0]
            sys.modules["antenv.axon_hooks"] = mod
            antenv.axon_hooks = mod
            try:
                sys.path.insert(0, "/root/.axon_site/trn_agent_boot")
                import trn_boot
                mod.set_axon_ntff_profile_hook(
                    trn_boot._ntff_profile_via_ctypes("/opt/axon/libaxon_pjrt.so"))
            except Exception:
                pass
    except Exception:
        pass


_env_setup()

import ml_dtypes  # noqa: E402

bf16 = ml_dtypes.bfloat16


def _bucket(x):
    return BUCKETS[np.searchsorted(BUCKETS, x)]


def _wrap_idx(v):
    n = len(v)
    assert n % 16 == 0
    a = np.asarray(v, dtype=np.int16).reshape(n // 16, 16).T
    return np.tile(a, (8, 1))


def build_host(inputs):
    el = np.asarray(inputs["edge_list"])
    src = el[:, 0].astype(np.int64)
    dst = el[:, 1].astype(np.int64)
    rel = el[:, 2].astype(np.int64)
    h_index = np.asarray(inputs["h_index"])
    r_index = np.asarray(inputs["r_index"])
    query_emb = np.asarray(inputs["query_emb"], np.float64)
    h0 = h_index[:, 0].astype(np.int64)
    r0 = r_index[:, 0].astype(np.int64)
    query = query_emb[r0]

    notrunc = bool(os.environ.get("NBF_NOTRUNC"))
    T = np.zeros((B, N), dtype=bool)
    T[np.arange(B), h0] = True
    touched = []
    for l in range(L):
        touched.append(np.ones(N, bool) if notrunc else T.any(0).copy())
        if l < L - 1:
            for q in range(B):
                T[q, dst[T[q, src]]] = True

    seed_nodes = np.unique(h0)
    pat = {}
    for n in seed_nodes:
        p = np.zeros(NQ * D)
        for q in range(B):
            if h0[q] == n:
                p[q * D:(q + 1) * D] += query[q]
        pat[int(n)] = p
    is_seed = np.zeros(N, dtype=bool)
    is_seed[seed_nodes] = True
    bnd_entry_of = {int(n): ZERO_ENTRY + 1 + j
                    for j, n in enumerate(seed_nodes)}
    NRELE = ZERO_ENTRY + 1 + len(seed_nodes)

    indeg = np.bincount(dst, minlength=N)
    degree = indeg.astype(np.float64) + 1.0
    scale = np.log(degree)
    scale = scale / scale.mean()
    iscale = 1.0 / np.clip(scale, 1e-2, None)
    rcnt = 1.0 / degree

    e_by_dst = np.argsort(dst, kind="stable")
    dst_s = dst[e_by_dst]

    pos_prev = np.empty(N, dtype=np.int64)
    for c in range(NCORES):
        n0 = c * NPC0
        pos_prev[n0:n0 + NPC0] = c * NPC + np.arange(NPC0)
    pos_init = pos_prev.copy()

    CAP = NPC - NSEEDN
    shared_layers = []
    percore = [dict(idxh=[], relc=[], idxp=[], helpers=[], corr=[],
                    _prev_local=np.arange(NPC)) for _ in range(NCORES)]
    pos_by_layer = [pos_init]

    for l in range(L):
        act = touched[l][src]
        k_glob = np.bincount(dst[act], minlength=N)

        core_stat = []
        for c in range(NCORES):
            n0 = c * NPC0
            k = np.zeros(NPC, dtype=np.int64)
            k[:NPC0] = k_glob[n0:n0 + NPC0]
            sd = np.zeros(NPC, dtype=bool)
            sd[:NPC0] = is_seed[n0:n0 + NPC0]
            actn = (k > 0) | sd
            core_stat.append(dict(k=k, sd=sd, actn=actn))

        wseed = 2
        for st in core_stat:
            if st["sd"].any():
                wseed = max(wseed, int(_bucket(st["k"][st["sd"]].max() + 1)))

        ncl = len(BUCKETS)
        Cmat = np.zeros((NCORES, ncl), dtype=np.int64)
        for c in range(NCORES):
            st = core_stat[c]
            nid = np.arange(NPC)
            seedn = nid[st["sd"]]
            nonseed_act = nid[st["actn"] & ~st["sd"]]
            inact = nid[~st["actn"]][::-1]
            nb = NSEEDN - len(seedn)
            borrow = list(inact[:nb])
            if len(borrow) < nb:
                cand = nonseed_act[np.argsort(st["k"][nonseed_act],
                                              kind="stable")]
                need = nb - len(borrow)
                take = [int(p) for p in cand[:need]]
                assert all(st["k"][p] < wseed for p in take)
                borrow += take
            st["region"] = list(seedn) + borrow
            rem = np.setdiff1d(nonseed_act,
                               np.array(borrow, dtype=np.int64))
            st["place"] = rem
            st["wb"] = np.searchsorted(BUCKETS, st["k"][rem])
            Cmat[c] = np.bincount(st["wb"], minlength=ncl)

        suf = Cmat[:, ::-1].cumsum(1)[:, ::-1]
        M = suf.max(0)
        counts = M - np.append(M[1:], 0)
        assert counts.min() >= 0 and M[0] <= CAP

        cls = [(int(BUCKETS[i]), int(counts[i])) for i in range(ncl)
               if counts[i] > 0]
        nact_nodes = NSEEDN + sum(n for _, n in cls)

        runs_all = []
        soff = 0
        noff = 0
        runs_all.append((soff, noff, NSEEDN, wseed))
        soff += NSEEDN * wseed
        noff += NSEEDN
        for w, nn_ in cls:
            runs_all.append((soff, noff, nn_, w))
            soff += nn_ * w
            noff += nn_
        S = soff

        tiles = []
        cur = None
        for (rs, rn, nseg_all, w) in runs_all:
            s = 0
            while s < nseg_all:
                node = rn + s
                strip_i = next(i for i, (a, b) in enumerate(STRIPS)
                               if a <= node < b)
                strip_end = STRIPS[strip_i][1]
                if cur is not None and (cur[3] != strip_i
                                        or cur[1] + w > GTILE):
                    tiles.append(cur)
                    cur = None
                if cur is None:
                    cur = [rs + s * w, 0, [], strip_i]
                take = min(nseg_all - s, strip_end - node,
                           (GTILE - cur[1]) // w)
                assert take >= 1
                cur[2].append((cur[1], node, take, w))
                cur[1] += take * w
                s += take
        if cur is not None:
            tiles.append(cur)
        tiles_f = []
        xoff = 0
        for (ts, nv, truns, strip_i) in tiles:
            nvp = ((nv + 127) // 128) * 128
            tiles_f.append(dict(soff=ts, nv=nv, nvp=nvp, runs=truns,
                                strip=strip_i, xoff=xoff))
            xoff += nvp
        X_l = xoff

        shared_layers.append(dict(wseed=wseed, cls=cls,
                                  nact_nodes=nact_nodes, S=S,
                                  tiles=tiles_f, X=X_l))

        pos_cur = np.empty(N, dtype=np.int64)
        for c in range(NCORES):
            n0 = c * NPC0
            st = core_stat[c]
            k, sd = st["k"], st["sd"]
            nid = np.arange(NPC)
            order_desc = np.argsort(-st["wb"], kind="stable")
            plist = st["place"][order_desc]
            pbuck = st["wb"][order_desc]
            placed_by_class = {i: [] for i in range(ncl)}
            ptr = 0
            for i in range(ncl - 1, -1, -1):
                cnt = int(counts[i])
                while cnt > 0 and ptr < len(plist):
                    assert pbuck[ptr] <= i
                    placed_by_class[i].append(int(plist[ptr]))
                    ptr += 1
                    cnt -= 1
            assert ptr == len(plist)
            used = set(st["region"]) | set(int(p) for p in plist)
            dummy_pool = [int(p) for p in nid[::-1] if int(p) not in used
                          and not st["actn"][p]]
            di = 0
            node_order = list(st["region"])
            for i in range(ncl):
                need = int(counts[i]) - len(placed_by_class[i])
                fill = dummy_pool[di:di + need]
                di += need
                assert len(fill) == need
                used.update(fill)
                node_order.extend(placed_by_class[i] + fill)
            tail = [int(p) for p in nid if int(p) not in used]
            node_order = np.array(node_order + tail, dtype=np.int64)
            assert len(node_order) == NPC
            pos_local = np.empty(NPC, dtype=np.int64)
            pos_local[node_order] = np.arange(NPC)
            pos_cur[n0:n0 + NPC0] = c * NPC + pos_local[:NPC0]

            w_seq = np.empty(nact_nodes, dtype=np.int64)
            w_seq[:NSEEDN] = wseed
            o = NSEEDN
            for w, nn_ in cls:
                w_seq[o:o + nn_] = w
                o += nn_
            off = np.zeros(nact_nodes + 1, dtype=np.int64)
            np.cumsum(w_seq, out=off[1:])
            assert off[-1] == S

            idxh = np.full(S, ONES_GPOS, dtype=np.int64)
            relc = np.full(S, ZERO_ENTRY, dtype=np.int64)

            rank_of_local = np.full(NPC, -1, dtype=np.int64)
            rank_of_local[node_order[:nact_nodes]] = np.arange(nact_nodes)
            lo = np.searchsorted(dst_s, n0)
            hi = np.searchsorted(dst_s, n0 + NPC0)
            ee = e_by_dst[lo:hi]
            ee = ee[act[ee]]
            if len(ee):
                rk = rank_of_local[dst[ee] - n0]
                assert (rk >= 0).all()
                o2 = np.argsort(rk, kind="stable")
                ee = ee[o2]
                rks = rk[o2]
                grp_start = np.searchsorted(rks, np.arange(nact_nodes))
                within = np.arange(len(ee)) - grp_start[rks]
                assert (within < w_seq[rks]).all()
                slotpos = off[rks] + within
                idxh[slotpos] = pos_prev[src[ee]]
                relc[slotpos] = rel[ee]

            corr = np.zeros((2, 128, NSEEDN), dtype=np.float64)
            for irank in range(NSEEDN):
                ln = node_order[irank]
                if not sd[ln]:
                    continue
                n_old = n0 + ln
                p = pat[int(n_old)]
                kk = int(k[ln])
                pad0 = int(off[irank]) + kk
                pad1 = int(off[irank + 1])
                idxh[pad0:pad1] = ONES_GPOS
                relc[pad0:pad1] = bnd_entry_of[int(n_old)]
                npads = wseed - kk
                assert npads >= 1
                corr[0, :, irank] = (npads - 1) * p
                corr[1, :, irank] = (npads - 1) * p * p

            idxh_t = []
            relc_t = []
            for t in tiles_f:
                bh = np.full(t["nvp"], ONES_GPOS, dtype=np.int64)
                br = np.full(t["nvp"], ZERO_ENTRY, dtype=np.int64)
                for (toff, tnode, nseg, w) in t["runs"]:
                    base = off[tnode]
                    bh[toff:toff + nseg * w] = \
                        idxh[base:base + nseg * w].reshape(nseg, w).T.ravel()
                    br[toff:toff + nseg * w] = \
                        relc[base:base + nseg * w].reshape(nseg, w).T.ravel()
                idxh_t.append(bh)
                relc_t.append(br)

            hv = np.ones((3, NPC), dtype=np.float64)
            valid = node_order < NPC0
            ov = node_order[valid] + n0
            hv[0, np.nonzero(valid)[0]] = rcnt[ov]
            hv[1, np.nonzero(valid)[0]] = scale[ov]
            hv[2, np.nonzero(valid)[0]] = iscale[ov]

            prev_local = percore[c]["_prev_local"]
            idxp = prev_local[node_order]
            percore[c]["_prev_local"] = pos_local

            percore[c]["idxh"].append(
                np.concatenate(idxh_t) if idxh_t else np.zeros(0, np.int64))
            percore[c]["relc"].append(
                np.concatenate(relc_t) if relc_t else np.zeros(0, np.int64))
            percore[c]["idxp"].append(idxp)
            percore[c]["helpers"].append(hv)
            percore[c]["corr"].append(corr)

        pos_by_layer.append(pos_cur.copy())
        pos_prev = pos_cur

    return dict(shared=shared_layers, percore=percore,
                pos_by_layer=pos_by_layer, query=query, h0=h0,
                seed_nodes=seed_nodes, pat=pat, NRELE=NRELE,
                bnd_entry_of=bnd_entry_of, pos_init=pos_init,
                t_index=np.asarray(inputs["t_index"]))


def build_weights(inputs, S):
    rel_W = np.asarray(inputs["rel_W"], np.float64)
    rel_b = np.asarray(inputs["rel_b"], np.float64)
    lin_W = np.asarray(inputs["lin_W"], np.float64)
    lin_b = np.asarray(inputs["lin_b"], np.float64)
    query = S["query"]
    NRELE = S["NRELE"]

    # rel palette per layer: [L, NRELE, 128] float32 (token per rel entry)
    relpal = np.zeros((L, NRELE, 128), dtype=np.float32)
    for l in range(L):
        remb = (query @ rel_W[l] + rel_b[l]).reshape(B, 2 * R, D)
        relpal[l, :2 * R] = remb.transpose(1, 0, 2).reshape(2 * R, NQ * D)
        for n_old, eid in S["bnd_entry_of"].items():
            relpal[l, eid] = S["pat"][n_old]

    wbd = np.zeros((L, 13, 128, 128), dtype=bf16)
    bias = np.zeros((L, 128, 1), dtype=np.float32)
    for l in range(L):
        Wh = lin_W[l][:D]
        Wu = lin_W[l][D:].reshape(D, 4, 3, D)
        for g in range(4):
            for ks in range(3):
                blk = Wu[:, g, ks, :]
                m = np.zeros((128, 128))
                for q in range(NQ):
                    m[q * D:(q + 1) * D, q * D:(q + 1) * D] = blk
                wbd[l, g * 3 + ks] = m.astype(bf16)
        m = np.zeros((128, 128))
        for q in range(NQ):
            m[q * D:(q + 1) * D, q * D:(q + 1) * D] = Wh
        wbd[l, 12] = m.astype(bf16)
        for q in range(NQ):
            bias[l, q * D:(q + 1) * D, 0] = lin_b[l]

    # initial table (token-major) + ones rows
    itbl = np.zeros((NTBL, 128), dtype=bf16)
    for n_old, p in S["pat"].items():
        itbl[S["pos_init"][n_old]] = p.astype(bf16)
    itbl[NTOT:] = bf16(1.0)
    # initial slab (own-core boundary, init order, token-major): same for
    # all cores? no: slab0[c] = itbl rows [c*NPC:(c+1)*NPC]
    return dict(relpal=relpal, wbd=wbd, bias=bias, itbl=itbl,
                mlp=(np.asarray(inputs["mlp_W1"], np.float64),
                     np.asarray(inputs["mlp_b1"], np.float64),
                     np.asarray(inputs["mlp_W2"], np.float64),
                     np.asarray(inputs["mlp_b2"], np.float64)))


def build_program(S, nlayers=L):
    import concourse.tile as tile
    from concourse import bacc, mybir
    import concourse.tile_utils as tile_utils
    try:
        tile_utils.max_sbuf_usage = 210 * 1024
    except Exception:
        pass
    import contextlib

    shared = S["shared"]
    X_tot = max(sum(sl["X"] for sl in shared[:nlayers]), 16)
    NPP16 = NPC // 16

    nc = bacc.Bacc("TRN2", target_bir_lowering=False, debug=False,
                   num_devices=NCORES)
    dtb = mybir.dt.bfloat16
    dtf = mybir.dt.float32
    dti = mybir.dt.int16
    OP = mybir.AluOpType
    AF = mybir.ActivationFunctionType

    idxh_d = nc.dram_tensor("idxh", [128, X_tot // 16], dti,
                            kind="ExternalInput")
    rs_d = nc.dram_tensor("rs", [128, X_tot], dtb, kind="ExternalInput")
    idxp_d = nc.dram_tensor("idxp", [128, nlayers * NPP16], dti,
                            kind="ExternalInput")
    help_d = nc.dram_tensor("helpers", [nlayers, 3, 128, NPC], dtb,
                            kind="ExternalInput")
    corr_d = nc.dram_tensor("corr", [nlayers, 2, 128, NSEEDN], dtb,
                            kind="ExternalInput")
    wbd_d = nc.dram_tensor("wbd", [nlayers, 13, 128, 128], dtb,
                           kind="ExternalInput")
    bias_d = nc.dram_tensor("biasl", [nlayers, 128, 1], dtf,
                            kind="ExternalInput")
    itbl_d = nc.dram_tensor("itbl", [NTBL, 128], dtb, kind="ExternalInput")
    slab0_d = nc.dram_tensor("slab0", [NPC, 128], dtb, kind="ExternalInput")
    ident_d = nc.dram_tensor("ident", [128, 128], dtb, kind="ExternalInput")
    outh_d = nc.dram_tensor("outh", [128, NPC], dtb, kind="ExternalOutput")

    in_slab = nc.dram_tensor("in_slab", [NPC, 128], dtb)
    tbl = nc.dram_tensor("tblhbm", [NTBL, 128], dtb, addr_space="Shared")

    with tile.TileContext(nc) as tc:
        ctx = contextlib.ExitStack()
        with ctx, nc.allow_low_precision(reason="bf16 stats by design"):
            pidx = ctx.enter_context(tc.tile_pool(name="pidx", bufs=3))
            pg = ctx.enter_context(tc.tile_pool(name="pg", bufs=2))
            pr = ctx.enter_context(tc.tile_pool(name="pr", bufs=2))
            pm = ctx.enter_context(tc.tile_pool(name="pm", bufs=2))
            plvl = ctx.enter_context(tc.tile_pool(name="plvl", bufs=2))
            pgrid = ctx.enter_context(tc.tile_pool(name="pgrid", bufs=1))
            phlp = ctx.enter_context(tc.tile_pool(name="phlp", bufs=1))
            pw = ctx.enter_context(tc.tile_pool(name="pw", bufs=1))
            phid = ctx.enter_context(tc.tile_pool(name="phid", bufs=1))
            pt = ctx.enter_context(tc.tile_pool(name="pt", bufs=2))
            pstg = ctx.enter_context(tc.tile_pool(name="pstg", bufs=2))
            ppsum = ctx.enter_context(tc.tile_pool(name="ppsum", bufs=2,
                                                   space="PSUM"))
            ppsT = ctx.enter_context(tc.tile_pool(name="ppsT", bufs=2,
                                                  space="PSUM"))

            nc.sync.dma_start(out=tbl[:], in_=itbl_d[:])
            ident = pw.tile([128, 128], dtb, tag="ident")
            nc.sync.dma_start(out=ident[:], in_=ident_d[:])
            gather_src = tbl[IDX_BASE:NTBL]

            for l in range(nlayers):
                sl = shared[l]
                xbase = sum(s2["X"] for s2 in shared[:l])
                wbd = pw.tile([128, 13, 128], dtb, tag="wbd")
                nc.sync.dma_start(
                    out=wbd[:], in_=wbd_d[l].rearrange("k p f -> p k f"))
                biasv = pw.tile([128, 1], dtf, tag="biasv")
                nc.sync.dma_start(out=biasv[:], in_=bias_d[l])
                corrt = pw.tile([128, 2, NSEEDN], dtb, tag="corrt")
                nc.sync.dma_start(out=corrt[:],
                                  in_=corr_d[l].rearrange("k p f -> p k f"))

                # hprev reorder gather (from prev slab, HBM source)
                hp = phid.tile([128, NPC], dtb, tag="hp", name="hp")
                ipt = pidx.tile([128, NPP16], dti, tag="ip")
                nc.sync.dma_start(
                    out=ipt[:], in_=idxp_d[:, l * NPP16:(l + 1) * NPP16])
                nc.gpsimd.dma_gather(
                    out_ap=hp[:].rearrange("p (c n) -> p c n", c=1),
                    in_ap=(slab0_d if l == 0 else in_slab)[:],
                    idxs_ap=ipt[:], num_idxs=NPC, num_idxs_reg=NPC,
                    elem_size=128, transpose=True, single_packet=False)

                for si, (s0, s1) in enumerate(STRIPS):
                    ns = s1 - s0
                    g_sum = pgrid.tile([128, 2176], dtb, tag="g_sum",
                                       name="g_sum")[:, :ns]
                    g_max = pgrid.tile([128, 2176], dtb, tag="g_max",
                                       name="g_max")[:, :ns]
                    g_min = pgrid.tile([128, 2176], dtb, tag="g_min",
                                       name="g_min")[:, :ns]
                    g_sq = pgrid.tile([128, 2176], dtb, tag="g_sq",
                                      name="g_sq")[:, :ns]
                    act_end = max(0, min(sl["nact_nodes"], s1) - s0)
                    if act_end < ns:
                        for g in (g_sum, g_max, g_min, g_sq):
                            nc.vector.memset(g[:, act_end:ns], 0.0)

                    for t in [t for t in sl["tiles"] if t["strip"] == si]:
                        nv, nvp = t["nv"], t["nvp"]
                        ih = pidx.tile([128, GTILE // 16], dti, tag="ih")
                        nc.sync.dma_start(
                            out=ih[:, :nvp // 16],
                            in_=idxh_d[:, (xbase + t["xoff"]) // 16:
                                       (xbase + t["xoff"] + nvp) // 16])
                        gt = pg.tile([128, GTILE], dtb, tag="gt")
                        nc.gpsimd.dma_gather(
                            out_ap=gt[:, :nvp].rearrange(
                                "p (c n) -> p c n", c=1),
                            in_ap=gather_src[:],
                            idxs_ap=ih[:, :nvp // 16],
                            num_idxs=nvp, num_idxs_reg=nvp, elem_size=128,
                            transpose=True, single_packet=False)
                        rt = pr.tile([128, GTILE], dtb, tag="rt")
                        nc.sync.dma_start(
                            out=rt[:, :nvp],
                            in_=rs_d[:, xbase + t["xoff"]:
                                     xbase + t["xoff"] + nvp])
                        nc.vector.tensor_tensor(out=gt[:, :nv],
                                                in0=gt[:, :nv],
                                                in1=rt[:, :nv], op=OP.mult)
                        msq = pm.tile([128, GTILE], dtb, tag="msq")
                        nc.scalar.activation(msq[:, :nv], gt[:, :nv],
                                             AF.Square)
                        for (toff, tnode, nseg, w) in t["runs"]:
                            noff = tnode - s0
                            _tree(nc, plvl, OP.add, gt, toff, nseg, w,
                                  g_sum, noff, mybir)
                            _tree(nc, plvl, OP.max, gt, toff, nseg, w,
                                  g_max, noff, mybir)
                            _tree(nc, plvl, OP.min, gt, toff, nseg, w,
                                  g_min, noff, mybir)
                            _tree(nc, plvl, OP.add, msq, toff, nseg, w,
                                  g_sq, noff, mybir)

                    if si == 0:
                        nc.vector.tensor_tensor(
                            out=g_sum[:, :NSEEDN], in0=g_sum[:, :NSEEDN],
                            in1=corrt[:, 0], op=OP.subtract)
                        nc.vector.tensor_tensor(
                            out=g_sq[:, :NSEEDN], in0=g_sq[:, :NSEEDN],
                            in1=corrt[:, 1], op=OP.subtract)
                        mm0 = NSEEDN
                    else:
                        mm0 = 0
                    nc.vector.tensor_scalar_max(g_max[:, mm0:],
                                                g_max[:, mm0:], 0.0)
                    nc.vector.tensor_scalar_min(g_min[:, mm0:],
                                                g_min[:, mm0:], 0.0)

                    hlp = phlp.tile([128, 3, 2176], dtb, tag="hlp",
                                    name="hlp")[:, :, :ns]
                    nc.sync.dma_start(out=hlp[:],
                                      in_=help_d[l, :, :, s0:s1].rearrange(
                                          "k p f -> p k f"))
                    nc.vector.tensor_tensor(out=g_sum[:], in0=g_sum[:],
                                            in1=hlp[:, 0], op=OP.mult)
                    nc.vector.tensor_tensor(out=g_sq[:], in0=g_sq[:],
                                            in1=hlp[:, 0], op=OP.mult)
                    msc = phlp.tile([128, 2176], dtb, tag="msc",
                                    name="msc")[:, :ns]
                    nc.scalar.activation(msc[:], g_sum[:], AF.Square)
                    nc.vector.tensor_tensor(out=g_sq[:], in0=g_sq[:],
                                            in1=msc[:], op=OP.subtract)
                    nc.vector.tensor_scalar_max(g_sq[:], g_sq[:], EPS)
                    nc.scalar.activation(g_sq[:], g_sq[:], AF.Sqrt)

                    hnew = phid.tile([128, 2176], dtb, tag="hn",
                                     name="hn")[:, :ns]
                    mt0 = 0
                    while mt0 < ns:
                        mt = min(512, ns - mt0)
                        ps = [ppsum.tile([128, 512], dtf, tag=f"ps{k2}",
                                         name=f"ps{k2}") for k2 in range(3)]
                        gl = [g_sum, g_max, g_min, g_sq]
                        for ks in range(3):
                            for g in range(4):
                                nc.tensor.matmul(
                                    ps[ks][:, :mt], wbd[:, g * 3 + ks, :],
                                    gl[g][:, mt0:mt0 + mt],
                                    start=(g == 0),
                                    stop=(g == 3 and ks != 0))
                        nc.tensor.matmul(ps[0][:, :mt], wbd[:, 12, :],
                                         hp[:, s0 + mt0:s0 + mt0 + mt],
                                         start=False, stop=True)
                        t1 = pt.tile([128, 512], dtb, tag="t1")
                        nc.vector.tensor_tensor(out=t1[:, :mt],
                                                in0=ps[1][:, :mt],
                                                in1=hlp[:, 1, mt0:mt0 + mt],
                                                op=OP.mult)
                        t2 = pt.tile([128, 512], dtb, tag="t2")
                        nc.vector.tensor_tensor(out=t2[:, :mt],
                                                in0=ps[2][:, :mt],
                                                in1=hlp[:, 2, mt0:mt0 + mt],
                                                op=OP.mult)
                        nc.vector.tensor_tensor(out=t1[:, :mt],
                                                in0=t1[:, :mt],
                                                in1=t2[:, :mt], op=OP.add)
                        nc.vector.tensor_tensor(out=t1[:, :mt],
                                                in0=t1[:, :mt],
                                                in1=ps[0][:, :mt], op=OP.add)
                        nc.scalar.activation(hnew[:, mt0:mt0 + mt],
                                             t1[:, :mt], AF.Relu,
                                             bias=biasv[:])
                        mt0 += mt

                    if l == nlayers - 1:
                        nc.sync.dma_start(out=outh_d[:, s0:s1], in_=hnew[:])
                    else:
                        for rk in range(ns // 128):
                            psT = ppsT.tile([128, 128], dtb, tag="psT")
                            nc.tensor.transpose(
                                psT[:], hnew[:, rk * 128:(rk + 1) * 128],
                                ident[:])
                            stg = pstg.tile([128, 128], dtb, tag="stg")
                            nc.scalar.activation(stg[:], psT[:], AF.Copy)
                            nc.sync.dma_start(
                                out=in_slab[s0 + rk * 128:
                                            s0 + (rk + 1) * 128, :],
                                in_=stg[:])

                if l != nlayers - 1:
                    nc.gpsimd.collective_compute(
                        "AllGather", OP.bypass,
                        replica_groups=[list(range(NCORES))],
                        ins=[in_slab[:]],
                        outs=[tbl[0:NTOT].rearrange("(c n) d -> c n d",
                                                    c=NCORES)])

    nc.compile()
    return nc


def _tree(nc, plvl, op, srcbuf, toff, nseg, w, grid, noff, mybir):
    dtb = mybir.dt.bfloat16
    if w == 1:
        nc.vector.tensor_copy(grid[:, noff:noff + nseg],
                              srcbuf[:, toff:toff + nseg])
        return
    cur, cof, m = srcbuf, toff, w
    while m > 1:
        h = m // 2
        odd = m - 2 * h
        last = (h == 1 and odd == 0)
        if last:
            nxt, nof = grid, noff
        else:
            nxt = plvl.tile([128, GTILE // 2], dtb, tag="lvl")
            nof = 0
        nc.vector.tensor_tensor(
            out=nxt[:, nof:nof + h * nseg],
            in0=cur[:, cof:cof + h * nseg],
            in1=cur[:, cof + h * nseg:cof + 2 * h * nseg], op=op)
        if odd:
            if h == 1:
                nc.vector.tensor_tensor(
                    out=grid[:, noff:noff + nseg],
                    in0=nxt[:, nof:nof + nseg],
                    in1=cur[:, cof + 2 * h * nseg:cof + (2 * h + 1) * nseg],
                    op=op)
                return
            nc.vector.tensor_tensor(
                out=nxt[:, nof:nof + nseg],
                in0=nxt[:, nof:nof + nseg],
                in1=cur[:, cof + 2 * h * nseg:cof + (2 * h + 1) * nseg],
                op=op)
        cur, cof, m = nxt, nof, h


_RUN_STATE = {}


def kernel(**inputs):
    from concourse.bass_utils import run_bass_kernel_spmd

    S = build_host(inputs)
    W = build_weights(inputs, S)
    nlayers = int(os.environ.get("NBF_LAYERS", L))
    nc = build_program(S, nlayers=nlayers)

    shared = S["shared"]
    X_tot = max(sum(sl["X"] for sl in shared[:nlayers]), 16)
    NPP16 = NPC // 16

    in_maps = []
    for c in range(NCORES):
        pc = S["percore"][c]
        ih = np.concatenate(pc["idxh"][:nlayers]) if X_tot > 16 else \
            np.full(16, ONES_GPOS, np.int64)
        ihp = np.full(X_tot, ONES_GPOS, np.int64)
        ihp[:len(ih)] = ih
        ihw = _wrap_idx(ihp - IDX_BASE)
        rc = np.concatenate(pc["relc"][:nlayers]) if X_tot > 16 else \
            np.full(16, ZERO_ENTRY, np.int64)
        rcp = np.full(X_tot, ZERO_ENTRY, np.int64)
        rcp[:len(rc)] = rc
        # expand rel stream: [128, X_tot] bf16, layer-sliced palette
        rsb = np.zeros((128, X_tot), dtype=bf16)
        xo = 0
        for l in range(nlayers):
            nl = len(pc["relc"][l])
            if nl:
                rsb[:, xo:xo + nl] = \
                    W["relpal"][l][rcp[xo:xo + nl]].T.astype(bf16)
            xo += nl
        ipw = np.concatenate([_wrap_idx(pc["idxp"][l])
                              for l in range(nlayers)], axis=1)
        helpers = np.stack([pc["helpers"][l] for l in range(nlayers)]
                           ).astype(bf16)
        helpers = np.broadcast_to(helpers[:, :, None, :],
                                  (nlayers, 3, 128, NPC)).copy()
        corr = np.stack([pc["corr"][l] for l in range(nlayers)]).astype(bf16)
        in_maps.append(dict(
            idxh=ihw, rs=rsb, idxp=ipw, helpers=helpers, corr=corr,
            wbd=np.ascontiguousarray(W["wbd"][:nlayers]),
            biasl=W["bias"][:nlayers],
            itbl=W["itbl"],
            slab0=np.ascontiguousarray(W["itbl"][c * NPC:(c + 1) * NPC]),
            ident=np.eye(128, dtype=bf16),
        ))

    res = run_bass_kernel_spmd(nc, in_maps, core_ids=list(range(NCORES)),
                               trace=bool(os.environ.get("NBF_TRACE")))
    _RUN_STATE["exec_time_ns"] = res.exec_time_ns

    posL = S["pos_by_layer"][nlayers]
    hid = np.zeros((N, NQ * D), dtype=np.float64)
    outs = []
    for c in range(NCORES):
        o = np.asarray(res.results[c]["outh"])
        if o.dtype != bf16:
            o = o.view(bf16)
        outs.append(o.astype(np.float64))
    core_idx = posL // NPC
    col = posL % NPC
    for c in range(NCORES):
        m = core_idx == c
        hid[m] = outs[c][:, col[m]].T

    mlp_W1, mlp_b1, mlp_W2, mlp_b2 = W["mlp"]
    t_index = S["t_index"]
    query = S["query"]
    Kk = t_index.shape[1]
    score = np.zeros((B, Kk), dtype=np.float32)
    for q in range(B):
        feat = np.concatenate(
            [hid[t_index[q], q * D:(q + 1) * D], np.tile(query[q], (Kk, 1))],
            -1)
        hdd = np.maximum(feat @ mlp_W1 + mlp_b1, 0)
        score[q] = ((hdd @ mlp_W2 + mlp_b2)[:, 0]).astype(np.float32)
    return score



# revision 4
# speedup vs baseline: 6.6957x; 6.6957x over previous
"""Trainium2 Bass kernel v4 for NBFNet: exact backward-dependency-cone
truncation (score only needs hidden[t_idx]; restrict each layer to
FW_{l+1} & N_{l+1} nodes), host-side generic evolution for untouched
nodes, compact per-layer AllGather tables, single-strip DVE trees."""
import os
import sys
import types
import numpy as np

N = 50000
R = 100
D = 32
L = 6
B = 4
NQ = 4
EPS = 1e-6
NCORES = 8
NPC0 = N // NCORES
NSEEDN = 4
ZERO_ENTRY = 2 * R          # rel palette: [0,2R) rels, 2R zero, 2R+1.. bnd
ZERO_ROW = 0
ONES_ROW = 1
PAT_BASE = 2
SROWS = 16
BUCKETS = np.array([1, 2, 3, 4, 6, 8, 10, 12, 16, 20, 24, 32, 48, 64, 96,
                    128, 192, 256, 384, 512], dtype=np.int64)


def _env_setup():
    if "/opt/trn_rl_repo" not in sys.path:
        sys.path.insert(0, "/opt/trn_rl_repo")
    try:
        import antenv  # noqa
        if "antenv.axon_hooks" not in sys.modules:
            hook = [None]
            mod = types.ModuleType("antenv.axon_hooks")
            mod.set_axon_ntff_profile_hook = lambda h: hook.__setitem__(0, h)
            mod.get_axon_ntff_profile_hook = lambda: hook[0]
            sys.modules["antenv.axon_hooks"] = mod
            antenv.axon_hooks = mod
            try:
                sys.path.insert(0, "/root/.axon_site/trn_agent_boot")
                import trn_boot
                mod.set_axon_ntff_profile_hook(
                    trn_boot._ntff_profile_via_ctypes("/opt/axon/libaxon_pjrt.so"))
            except Exception:
                pass
    except Exception:
        pass


_env_setup()

import ml_dtypes  # noqa: E402

bf16 = ml_dtypes.bfloat16


def _bucket(x):
    return BUCKETS[np.searchsorted(BUCKETS, x)]


def _rup(x, m):
    return (int(x) + m - 1) // m * m


def _wrap_idx(v):
    n = len(v)
    assert n % 16 == 0
    a = np.asarray(v, dtype=np.int16).reshape(n // 16, 16).T
    return np.tile(a, (8, 1))


def build_host(inputs):
    el = np.asarray(inputs["edge_list"])
    src = el[:, 0].astype(np.int64)
    dst = el[:, 1].astype(np.int64)
    rel = el[:, 2].astype(np.int64)
    h_index = np.asarray(inputs["h_index"])
    r_index = np.asarray(inputs["r_index"])
    t_index = np.asarray(inputs["t_index"])
    query_emb = np.asarray(inputs["query_emb"], np.float64)
    lin_W = np.asarray(inputs["lin_W"], np.float64)
    lin_b = np.asarray(inputs["lin_b"], np.float64)
    h0 = h_index[:, 0].astype(np.int64)
    r0 = r_index[:, 0].astype(np.int64)
    query = query_emb[r0]

    # forward wavefront (value-based, per-query then union, as reference)
    T = np.zeros((B, N), dtype=bool)
    T[np.arange(B), h0] = True
    FW = []
    for l in range(L + 1):
        FW.append(T.any(0).copy())
        if l < L:
            for q in range(B):
                T[q, dst[T[q, src]]] = True

    # backward needed sets
    tgts = np.unique(t_index)
    Nl = [None] * (L + 1)
    m = np.zeros(N, dtype=bool)
    m[tgts] = True
    Nl[L] = m
    for l in range(L - 1, -1, -1):
        m2 = Nl[l + 1].copy()
        m2[src[Nl[l + 1][dst]]] = True
        Nl[l] = m2
    US = [FW[l + 1] & Nl[l + 1] for l in range(L)]

    # per-node constants (exact, host fp64)
    indeg = np.bincount(dst, minlength=N)
    degree = indeg.astype(np.float64) + 1.0
    scale = np.log(degree)
    scale = scale / scale.mean()
    iscale = 1.0 / np.clip(scale, 1e-2, None)
    rcnt = 1.0 / degree

    # seeds + boundary patterns
    seeds = np.unique(h0)
    pat = {}
    for n in seeds:
        p = np.zeros(NQ * D)
        for q in range(B):
            if h0[q] == n:
                p[q * D:(q + 1) * D] += query[q]
        pat[int(n)] = p
    is_seed = np.zeros(N, dtype=bool)
    is_seed[seeds] = True
    bnd_entry_of = {int(n): ZERO_ENTRY + 1 + j for j, n in enumerate(seeds)}
    pat_row_of = {int(n): PAT_BASE + j for j, n in enumerate(seeds)}
    NRELE = ZERO_ENTRY + 1 + len(seeds)

    # generic (untouched-node) evolution, host fp64, only for needed nodes
    need_g = np.zeros(N, dtype=bool)
    for l in range(1, L):
        need_g |= US[l] & ~US[l - 1]
    need_g |= Nl[L] & ~US[L - 1]
    need_g &= ~is_seed
    gnodes = np.nonzero(need_g)[0]
    gidx_of = np.full(N, -1, dtype=np.int64)
    gidx_of[gnodes] = np.arange(len(gnodes))
    gsnap = [np.zeros((len(gnodes), B, D))]
    gcur = gsnap[0]
    seps = np.sqrt(EPS)
    for l in range(L):
        Wh = lin_W[l][:D]
        Wu = lin_W[l][D:].reshape(D, 4, 3, D)
        Av = Wu[:, 3, 0, :].sum(0)
        Bv = Wu[:, 3, 1, :].sum(0)
        Cv = Wu[:, 3, 2, :].sum(0)
        std_term = seps * (Av[None, :] + scale[gnodes, None] * Bv[None, :]
                           + iscale[gnodes, None] * Cv[None, :])
        gcur = np.maximum(
            gcur @ Wh + std_term[:, None, :] + lin_b[l][None, None, :], 0.0)
        gsnap.append(gcur)

    # pre-pass: gext sizes (per-core new non-seed cols for layers >= 1)
    core_of = lambda n: n // NPC0  # noqa: E731
    gext_count = np.zeros(NCORES, dtype=np.int64)
    for l in range(1, L):
        new = US[l] & ~US[l - 1] & ~is_seed
        for n in np.nonzero(new)[0]:
            gext_count[core_of(n)] += 1
    GE = _rup(gext_count.max(), 16) if gext_count.max() else 16
    ag0 = SROWS + GE

    e_by_dst = np.argsort(dst, kind="stable")
    dst_s = dst[e_by_dst]

    prev_col = np.full(N, -1, dtype=np.int64)
    prev_Cp = 0
    agoff = [0] * (L + 1)   # agoff[l] = table offset of region used at layer l
    nxt = ag0

    layers = []
    percore = [dict(idxh=[], relc=[], idxp=[], helpers=[], corr=[],
                    gext_rows=[], colmap=[]) for _ in range(NCORES)]

    for l in range(L):
        e_act = FW[l][src] & US[l][dst]
        k_glob = np.bincount(dst[e_act], minlength=N)

        core_stat = []
        for c in range(NCORES):
            n0 = c * NPC0
            rng = np.arange(n0, n0 + NPC0)
            usn = rng[US[l][rng]]
            sd = usn[is_seed[usn]]
            core_stat.append(dict(n0=n0, usn=usn, sd=sd))

        wseed = 2
        for st in core_stat:
            if len(st["sd"]):
                wseed = max(wseed, int(_bucket(k_glob[st["sd"]].max() + 1)))

        ncl = len(BUCKETS)
        Cmat = np.zeros((NCORES, ncl), dtype=np.int64)
        Zv = np.zeros(NCORES, dtype=np.int64)
        for c, st in enumerate(core_stat):
            nonseed = st["usn"][~is_seed[st["usn"]]]
            act = nonseed[k_glob[nonseed] > 0]
            st["act"] = act
            st["zk"] = nonseed[k_glob[nonseed] == 0]
            st["wb"] = np.searchsorted(BUCKETS, k_glob[act])
            Cmat[c] = np.bincount(st["wb"], minlength=ncl)
            Zv[c] = len(st["zk"])

        suf = Cmat[:, ::-1].cumsum(1)[:, ::-1]
        M = suf.max(0)
        counts = M - np.append(M[1:], 0)
        Z = int(Zv.max())
        cls = [(int(BUCKETS[i]), int(counts[i])) for i in range(ncl)
               if counts[i] > 0]
        A = NSEEDN + int(counts.sum())
        C = A + Z
        Cp = _rup(max(C, 1), 128)

        w_seq = np.zeros(A, dtype=np.int64)
        w_seq[:NSEEDN] = wseed
        o = NSEEDN
        for w, nn_ in cls:
            w_seq[o:o + nn_] = w
            o += nn_
        off = np.zeros(A + 1, dtype=np.int64)
        np.cumsum(w_seq, out=off[1:])
        S = int(off[-1])
        Xp = _rup(max(S, 16), 128)

        runs = [(0, 0, NSEEDN, wseed)]
        soff, noff = NSEEDN * wseed, NSEEDN
        for w, nn_ in cls:
            runs.append((soff, noff, nn_, w))
            soff += nn_ * w
            noff += nn_

        layers.append(dict(wseed=wseed, cls=cls, A=A, C=C, Cp=Cp, S=S,
                           Xp=Xp, runs=runs, agoff=agoff[l]))

        new_col = np.full(N, -1, dtype=np.int64)
        for c, st in enumerate(core_stat):
            n0 = c * NPC0
            # non-US nodes of this core usable as dummies
            rngl = np.arange(n0, n0 + NPC0)
            dummies = rngl[~US[l][rngl]][::-1]
            di = 0

            region = list(st["sd"])
            while len(region) < NSEEDN:
                region.append(int(dummies[di]))
                di += 1

            order_desc = np.argsort(-st["wb"], kind="stable")
            plist = st["act"][order_desc]
            pbuck = st["wb"][order_desc]
            placed = {i: [] for i in range(ncl)}
            ptr = 0
            for i in range(ncl - 1, -1, -1):
                cnt = int(counts[i])
                while cnt > 0 and ptr < len(plist):
                    assert pbuck[ptr] <= i
                    placed[i].append(int(plist[ptr]))
                    ptr += 1
                    cnt -= 1
            assert ptr == len(plist)
            node_order = list(region)
            for i in range(ncl):
                needi = int(counts[i]) - len(placed[i])
                fill = [int(dummies[di + j]) for j in range(needi)]
                di += needi
                node_order.extend(placed[i] + fill)
            zlist = list(st["zk"])
            while len(zlist) < Z:
                zlist.append(int(dummies[di]))
                di += 1
            node_order.extend(zlist)
            while len(node_order) < Cp:
                node_order.append(int(dummies[di]))
                di += 1
            node_order = np.array(node_order, dtype=np.int64)
            assert len(node_order) == Cp

            # slot stream
            idxh = np.full(Xp, ONES_ROW, dtype=np.int64)
            relc = np.full(Xp, ZERO_ENTRY, dtype=np.int64)
            rank_of = np.full(N, -1, dtype=np.int64)
            rank_of[node_order[:A]] = np.arange(A)
            lo = np.searchsorted(dst_s, n0)
            hi = np.searchsorted(dst_s, n0 + NPC0)
            ee = e_by_dst[lo:hi]
            ee = ee[e_act[ee]]
            if len(ee):
                rk = rank_of[dst[ee]]
                assert (rk >= 0).all()
                o2 = np.argsort(rk, kind="stable")
                ee = ee[o2]
                rks = rk[o2]
                grp = np.searchsorted(rks, np.arange(A))
                within = np.arange(len(ee)) - grp[rks]
                assert (within < w_seq[rks]).all()
                slotpos = off[rks] + within
                if l == 0:
                    spos = np.array([pat_row_of[int(s)] for s in src[ee]],
                                    dtype=np.int64)
                else:
                    pc = prev_col[src[ee]]
                    assert (pc >= 0).all()
                    spos = agoff[l] + (src[ee] // NPC0) * prev_Cp + pc
                idxh[slotpos] = spos
                relc[slotpos] = rel[ee]

            corr = np.zeros((2, 128, NSEEDN), dtype=np.float64)
            for irank in range(NSEEDN):
                n_ = int(node_order[irank])
                if not (is_seed[n_] and US[l][n_]):
                    continue
                p = pat[n_]
                kk = int(k_glob[n_])
                pad0 = int(off[irank]) + kk
                pad1 = int(off[irank + 1])
                idxh[pad0:pad1] = ONES_ROW
                relc[pad0:pad1] = bnd_entry_of[n_]
                npads = wseed - kk
                assert npads >= 1
                corr[0, :, irank] = (npads - 1) * p
                corr[1, :, irank] = (npads - 1) * p * p

            # hp indices
            idxp = np.full(Cp, ONES_ROW, dtype=np.int64)
            for j, n_ in enumerate(node_order):
                n_ = int(n_)
                if not US[l][n_]:
                    continue
                if l == 0:
                    idxp[j] = pat_row_of[n_] if is_seed[n_] else ZERO_ROW
                elif prev_col[n_] >= 0:
                    idxp[j] = agoff[l] + (n_ // NPC0) * prev_Cp + prev_col[n_]
                else:
                    assert not is_seed[n_]
                    gi = gidx_of[n_]
                    assert gi >= 0
                    row = SROWS + len(percore[c]["gext_rows"])
                    percore[c]["gext_rows"].append(
                        gsnap[l][gi].reshape(NQ * D))
                    idxp[j] = row

            hv = np.ones((3, Cp), dtype=np.float64)
            usm = US[l][node_order]
            hv[0, usm] = rcnt[node_order[usm]]
            hv[1, usm] = scale[node_order[usm]]
            hv[2, usm] = iscale[node_order[usm]]

            new_col[node_order[usm]] = np.nonzero(usm)[0]

            percore[c]["idxh"].append(idxh)
            percore[c]["relc"].append(relc)
            percore[c]["idxp"].append(idxp)
            percore[c]["helpers"].append(hv)
            percore[c]["corr"].append(corr)
            percore[c]["colmap"].append(
                dict((int(n2), int(j2)) for j2, n2 in enumerate(node_order)
                     if US[l][n2]))

        prev_col = new_col
        prev_Cp = Cp
        if l < L - 1:
            agoff[l + 1] = nxt
            nxt += NCORES * Cp
    TROWS = nxt
    assert TROWS <= 32767, TROWS

    return dict(layers=layers, percore=percore, query=query, seeds=seeds,
                pat=pat, NRELE=NRELE, bnd_entry_of=bnd_entry_of,
                GE=GE, TROWS=TROWS, t_index=t_index, US=US,
                gsnap=gsnap, gidx_of=gidx_of, is_seed=is_seed)


def build_weights(inputs, S):
    rel_W = np.asarray(inputs["rel_W"], np.float64)
    rel_b = np.asarray(inputs["rel_b"], np.float64)
    lin_W = np.asarray(inputs["lin_W"], np.float64)
    lin_b = np.asarray(inputs["lin_b"], np.float64)
    query = S["query"]
    NRELE = S["NRELE"]

    relpal = np.zeros((L, NRELE, 128), dtype=np.float32)
    for l in range(L):
        remb = (query @ rel_W[l] + rel_b[l]).reshape(B, 2 * R, D)
        relpal[l, :2 * R] = remb.transpose(1, 0, 2).reshape(2 * R, NQ * D)
        for n_old, eid in S["bnd_entry_of"].items():
            relpal[l, eid] = S["pat"][n_old]

    wbd = np.zeros((L, 13, 128, 128), dtype=bf16)
    bias = np.zeros((L, 128, 1), dtype=np.float32)
    for l in range(L):
        Wh = lin_W[l][:D]
        Wu = lin_W[l][D:].reshape(D, 4, 3, D)
        for g in range(4):
            for ks in range(3):
                blk = Wu[:, g, ks, :]
                m = np.zeros((128, 128))
                for q in range(NQ):
                    m[q * D:(q + 1) * D, q * D:(q + 1) * D] = blk
                wbd[l, g * 3 + ks] = m.astype(bf16)
        m = np.zeros((128, 128))
        for q in range(NQ):
            m[q * D:(q + 1) * D, q * D:(q + 1) * D] = Wh
        wbd[l, 12] = m.astype(bf16)
        for q in range(NQ):
            bias[l, q * D:(q + 1) * D, 0] = lin_b[l]

    return dict(relpal=relpal, wbd=wbd, bias=bias,
                mlp=(np.asarray(inputs["mlp_W1"], np.float64),
                     np.asarray(inputs["mlp_b1"], np.float64),
                     np.asarray(inputs["mlp_W2"], np.float64),
                     np.asarray(inputs["mlp_b2"], np.float64)))


def build_program(S):
    import concourse.tile as tile
    from concourse import bacc, mybir
    import contextlib

    layers = S["layers"]
    XT = sum(sl["Xp"] for sl in layers)
    CT = sum(sl["Cp"] for sl in layers)
    Cpmax = max(sl["Cp"] for sl in layers)
    Xpmax = max(sl["Xp"] for sl in layers)
    TROWS = S["TROWS"]
    GE = S["GE"]
    CpL = layers[L - 1]["Cp"]

    nc = bacc.Bacc("TRN2", target_bir_lowering=False, debug=False,
                   num_devices=NCORES)
    dtb = mybir.dt.bfloat16
    dtf = mybir.dt.float32
    dti = mybir.dt.int16
    OP = mybir.AluOpType
    AF = mybir.ActivationFunctionType

    idxh_d = nc.dram_tensor("idxh", [128, XT // 16], dti,
                            kind="ExternalInput")
    idxp_d = nc.dram_tensor("idxp", [128, CT // 16], dti,
                            kind="ExternalInput")
    rs_d = nc.dram_tensor("rs", [128, XT], dtb, kind="ExternalInput")
    help_d = nc.dram_tensor("helpers", [128, 3 * CT], dtb,
                            kind="ExternalInput")
    corr_d = nc.dram_tensor("corr", [128, L * 2 * NSEEDN], dtb,
                            kind="ExternalInput")
    wbd_d = nc.dram_tensor("wbd", [L, 13, 128, 128], dtb,
                           kind="ExternalInput")
    bias_d = nc.dram_tensor("biasl", [L, 128, 1], dtf, kind="ExternalInput")
    tstat_d = nc.dram_tensor("tstat", [SROWS + GE, 128], dtb,
                             kind="ExternalInput")
    ident_d = nc.dram_tensor("ident", [128, 128], dtb, kind="ExternalInput")
    outh_d = nc.dram_tensor("outh", [128, CpL], dtb, kind="ExternalOutput")

    tbl = nc.dram_tensor("tblhbm", [TROWS, 128], dtb, addr_space="Shared")
    slabs = [nc.dram_tensor(f"slab{i}", [Cpmax, 128], dtb) for i in range(2)]

    with tile.TileContext(nc) as tc:
        ctx = contextlib.ExitStack()
        with ctx, nc.allow_low_precision(reason="bf16 stats by design"):
            pw = ctx.enter_context(tc.tile_pool(name="pw", bufs=1))
            pg = ctx.enter_context(tc.tile_pool(name="pg", bufs=2))
            pm = ctx.enter_context(tc.tile_pool(name="pm", bufs=2))
            plvl = ctx.enter_context(tc.tile_pool(name="plvl", bufs=2))
            pgrid = ctx.enter_context(tc.tile_pool(name="pgrid", bufs=2))
            phid = ctx.enter_context(tc.tile_pool(name="phid", bufs=2))
            pt = ctx.enter_context(tc.tile_pool(name="pt", bufs=2))
            pstg = ctx.enter_context(tc.tile_pool(name="pstg", bufs=2))
            ppsum = ctx.enter_context(tc.tile_pool(name="ppsum", bufs=2,
                                                   space="PSUM"))
            ppsT = ctx.enter_context(tc.tile_pool(name="ppsT", bufs=2,
                                                  space="PSUM"))

            nc.sync.dma_start(out=tbl[0:SROWS + GE], in_=tstat_d[:])
            ident = pw.tile([128, 128], dtb, tag="ident")
            nc.sync.dma_start(out=ident[:], in_=ident_d[:])
            ihx = pw.tile([128, XT // 16], dti, tag="ihx")
            nc.sync.dma_start(out=ihx[:], in_=idxh_d[:])
            ipx = pw.tile([128, CT // 16], dti, tag="ipx")
            nc.sync.dma_start(out=ipx[:], in_=idxp_d[:])
            rsx = pw.tile([128, XT], dtb, tag="rsx")
            nc.sync.dma_start(out=rsx[:], in_=rs_d[:])
            hlpx = pw.tile([128, 3 * CT], dtb, tag="hlpx")
            nc.sync.dma_start(out=hlpx[:], in_=help_d[:])
            corx = pw.tile([128, L * 2 * NSEEDN], dtb, tag="corx")
            nc.sync.dma_start(out=corx[:], in_=corr_d[:])
            wbdx = pw.tile([128, L * 13, 128], dtb, tag="wbdx")
            nc.sync.dma_start(
                out=wbdx[:],
                in_=wbd_d[:].rearrange("l k p f -> p (l k) f"))
            biasx = pw.tile([128, L], dtf, tag="biasx")
            nc.sync.dma_start(out=biasx[:],
                              in_=bias_d[:].rearrange("l p f -> p (l f)"))

            xoff = 0
            coff = 0
            for l in range(L):
                sl = layers[l]
                Cp, Xp, A, Z = sl["Cp"], sl["Xp"], sl["A"], sl["C"] - sl["A"]
                wbd = wbdx[:, l * 13:(l + 1) * 13, :]
                biasv = biasx[:, l:l + 1]
                corrt = corx[:, l * 2 * NSEEDN:(l + 1) * 2 * NSEEDN]\
                    .rearrange("p (k f) -> p k f", k=2)
                hlp = hlpx[:, 3 * coff:3 * coff + 3 * Cp]\
                    .rearrange("p (k f) -> p k f", k=3)

                hp = phid.tile([128, Cpmax], dtb, tag="hp",
                               name="hp")[:, :Cp]
                nc.gpsimd.dma_gather(
                    out_ap=hp[:].rearrange("p (c n) -> p c n", c=1),
                    in_ap=tbl[:],
                    idxs_ap=ipx[:, coff // 16:(coff + Cp) // 16],
                    num_idxs=Cp, num_idxs_reg=Cp,
                    elem_size=128, transpose=True, single_packet=False)

                gt = pg.tile([128, Xpmax], dtb, tag="gt",
                             name="gt")[:, :Xp]
                nc.gpsimd.dma_gather(
                    out_ap=gt[:].rearrange("p (c n) -> p c n", c=1),
                    in_ap=tbl[:],
                    idxs_ap=ihx[:, xoff // 16:(xoff + Xp) // 16],
                    num_idxs=Xp, num_idxs_reg=Xp,
                    elem_size=128, transpose=True, single_packet=False)
                nc.vector.tensor_tensor(out=gt[:], in0=gt[:],
                                        in1=rsx[:, xoff:xoff + Xp],
                                        op=OP.mult)
                msq = pm.tile([128, Xpmax], dtb, tag="msq",
                              name="msq")[:, :Xp]
                nc.scalar.activation(msq[:], gt[:], AF.Square)

                g_sum = pgrid.tile([128, Cpmax], dtb, tag="g_sum",
                                   name="g_sum")[:, :Cp]
                g_max = pgrid.tile([128, Cpmax], dtb, tag="g_max",
                                   name="g_max")[:, :Cp]
                g_min = pgrid.tile([128, Cpmax], dtb, tag="g_min",
                                   name="g_min")[:, :Cp]
                g_sq = pgrid.tile([128, Cpmax], dtb, tag="g_sq",
                                  name="g_sq")[:, :Cp]
                if A < Cp:
                    for g in (g_sum, g_max, g_min, g_sq):
                        nc.vector.memset(g[:, A:Cp], 0.0)
                for (toff, tnode, nseg, w) in sl["runs"]:
                    _tree(nc, plvl, OP.add, gt, toff, nseg, w, g_sum,
                          tnode, mybir, Xpmax)
                    _tree(nc, plvl, OP.max, gt, toff, nseg, w, g_max,
                          tnode, mybir, Xpmax)
                    _tree(nc, plvl, OP.min, gt, toff, nseg, w, g_min,
                          tnode, mybir, Xpmax)
                    _tree(nc, plvl, OP.add, msq, toff, nseg, w, g_sq,
                          tnode, mybir, Xpmax)

                nc.vector.tensor_tensor(out=g_sum[:, :NSEEDN],
                                        in0=g_sum[:, :NSEEDN],
                                        in1=corrt[:, 0], op=OP.subtract)
                nc.vector.tensor_tensor(out=g_sq[:, :NSEEDN],
                                        in0=g_sq[:, :NSEEDN],
                                        in1=corrt[:, 1], op=OP.subtract)
                nc.vector.tensor_scalar_max(g_max[:, NSEEDN:],
                                            g_max[:, NSEEDN:], 0.0)
                nc.vector.tensor_scalar_min(g_min[:, NSEEDN:],
                                            g_min[:, NSEEDN:], 0.0)

                nc.vector.tensor_tensor(out=g_sum[:], in0=g_sum[:],
                                        in1=hlp[:, 0], op=OP.mult)
                nc.vector.tensor_tensor(out=g_sq[:], in0=g_sq[:],
                                        in1=hlp[:, 0], op=OP.mult)
                msc = pm.tile([128, Cpmax], dtb, tag="msc",
                              name="msc")[:, :Cp]
                nc.scalar.activation(msc[:], g_sum[:], AF.Square)
                nc.vector.tensor_tensor(out=g_sq[:], in0=g_sq[:],
                                        in1=msc[:], op=OP.subtract)
                nc.vector.tensor_scalar_max(g_sq[:], g_sq[:], EPS)
                nc.scalar.activation(g_sq[:], g_sq[:], AF.Sqrt)

                hnew = phid.tile([128, Cpmax], dtb, tag="hn",
                                 name="hn")[:, :Cp]
                mt0 = 0
                while mt0 < Cp:
                    mt = min(512, Cp - mt0)
                    ps = [ppsum.tile([128, 512], dtf, tag=f"ps{k2}",
                                     name=f"ps{k2}") for k2 in range(3)]
                    gl = [g_sum, g_max, g_min, g_sq]
                    for ks in range(3):
                        for g in range(4):
                            nc.tensor.matmul(
                                ps[ks][:, :mt], wbd[:, g * 3 + ks, :],
                                gl[g][:, mt0:mt0 + mt],
                                start=(g == 0),
                                stop=(g == 3 and ks != 0))
                    nc.tensor.matmul(ps[0][:, :mt], wbd[:, 12, :],
                                     hp[:, mt0:mt0 + mt],
                                     start=False, stop=True)
                    t1 = pt.tile([128, 512], dtb, tag="t1")
                    nc.vector.tensor_tensor(out=t1[:, :mt],
                                            in0=ps[1][:, :mt],
                                            in1=hlp[:, 1, mt0:mt0 + mt],
                                            op=OP.mult)
                    t2 = pt.tile([128, 512], dtb, tag="t2")
                    nc.vector.tensor_tensor(out=t2[:, :mt],
                                            in0=ps[2][:, :mt],
                                            in1=hlp[:, 2, mt0:mt0 + mt],
                                            op=OP.mult)
                    nc.vector.tensor_tensor(out=t1[:, :mt], in0=t1[:, :mt],
                                            in1=t2[:, :mt], op=OP.add)
                    nc.vector.tensor_tensor(out=t1[:, :mt], in0=t1[:, :mt],
                                            in1=ps[0][:, :mt], op=OP.add)
                    nc.scalar.activation(hnew[:, mt0:mt0 + mt], t1[:, :mt],
                                         AF.Relu, bias=biasv)
                    mt0 += mt

                if l == L - 1:
                    nc.sync.dma_start(out=outh_d[:], in_=hnew[:])
                else:
                    slab = slabs[l % 2]
                    for rk in range(Cp // 128):
                        psT = ppsT.tile([128, 128], dtb, tag="psT")
                        nc.tensor.transpose(
                            psT[:], hnew[:, rk * 128:(rk + 1) * 128],
                            ident[:])
                        stg = pstg.tile([128, 128], dtb, tag="stg")
                        nc.scalar.activation(stg[:], psT[:], AF.Copy)
                        nc.sync.dma_start(
                            out=slab[rk * 128:(rk + 1) * 128, :], in_=stg[:])
                    nc.gpsimd.collective_compute(
                        "AllGather", OP.bypass,
                        replica_groups=[list(range(NCORES))],
                        ins=[slab[0:Cp]],
                        outs=[tbl[layers[l + 1]["agoff"]:
                                  layers[l + 1]["agoff"] + NCORES * Cp]
                              .rearrange("(c n) d -> c n d", c=NCORES)])

                xoff += Xp
                coff += Cp

    nc.compile()
    return nc


def _tree(nc, plvl, op, srcbuf, toff, nseg, w, grid, noff, mybir, lvlw):
    dtb = mybir.dt.bfloat16
    if w == 1:
        nc.vector.tensor_copy(grid[:, noff:noff + nseg],
                              srcbuf[:, toff:toff + nseg])
        return
    cur, cof, m = srcbuf, toff, w
    while m > 1:
        h = m // 2
        odd = m - 2 * h
        last = (h == 1 and odd == 0)
        if last:
            nxt, nof = grid, noff
        else:
            nxt = plvl.tile([128, lvlw // 2], dtb, tag="lvl")
            nof = 0
        nc.vector.tensor_tensor(
            out=nxt[:, nof:nof + h * nseg],
            in0=cur[:, cof:cof + h * nseg],
            in1=cur[:, cof + h * nseg:cof + 2 * h * nseg], op=op)
        if odd:
            if h == 1:
                nc.vector.tensor_tensor(
                    out=grid[:, noff:noff + nseg],
                    in0=nxt[:, nof:nof + nseg],
                    in1=cur[:, cof + 2 * h * nseg:cof + (2 * h + 1) * nseg],
                    op=op)
                return
            nc.vector.tensor_tensor(
                out=nxt[:, nof:nof + nseg],
                in0=nxt[:, nof:nof + nseg],
                in1=cur[:, cof + 2 * h * nseg:cof + (2 * h + 1) * nseg],
                op=op)
        cur, cof, m = nxt, nof, h


_RUN_STATE = {}


def kernel(**inputs):
    from concourse.bass_utils import run_bass_kernel_spmd

    S = build_host(inputs)
    W = build_weights(inputs, S)
    nc = build_program(S)

    layers = S["layers"]
    XT = sum(sl["Xp"] for sl in layers)
    CT = sum(sl["Cp"] for sl in layers)
    GE = S["GE"]

    # static table rows (shared): zero, ones, patterns
    tstat_base = np.zeros((SROWS + GE, 128), dtype=bf16)
    tstat_base[ONES_ROW] = bf16(1.0)
    for n_old, p in S["pat"].items():
        tstat_base[PAT_BASE + (S["bnd_entry_of"][n_old] - ZERO_ENTRY - 1)] \
            = p.astype(bf16)

    in_maps = []
    for c in range(NCORES):
        pc = S["percore"][c]
        ihw = _wrap_idx(np.concatenate(pc["idxh"]))
        ipw = _wrap_idx(np.concatenate(pc["idxp"]))
        relc = np.concatenate(pc["relc"])
        rsb = np.zeros((128, XT), dtype=bf16)
        xo = 0
        for l in range(L):
            nl = len(pc["relc"][l])
            rsb[:, xo:xo + nl] = \
                W["relpal"][l][relc[xo:xo + nl]].T.astype(bf16)
            xo += nl
        hlpflat = np.concatenate(
            [pc["helpers"][l].reshape(-1) for l in range(L)])
        hlpb = np.broadcast_to(hlpflat[None, :].astype(bf16),
                               (128, 3 * CT)).copy()
        corrflat = np.concatenate(
            [pc["corr"][l].transpose(1, 0, 2).reshape(128, 2 * NSEEDN)
             for l in range(L)], axis=1).astype(bf16)
        tstat = tstat_base.copy()
        assert len(pc["gext_rows"]) <= GE
        for i, row in enumerate(pc["gext_rows"]):
            tstat[SROWS + i] = row.astype(bf16)
        in_maps.append(dict(
            idxh=ihw, idxp=ipw, rs=rsb, helpers=hlpb, corr=corrflat,
            wbd=np.ascontiguousarray(W["wbd"]), biasl=W["bias"],
            tstat=tstat, ident=np.eye(128, dtype=bf16)))

    res = run_bass_kernel_spmd(nc, in_maps, core_ids=list(range(NCORES)),
                               trace=bool(os.environ.get("NBF_TRACE")))
    _RUN_STATE["exec_time_ns"] = res.exec_time_ns

    t_index = S["t_index"]
    US5 = S["US"][L - 1]
    outs = []
    for c in range(NCORES):
        o = np.asarray(res.results[c]["outh"])
        if o.dtype != bf16:
            o = o.view(bf16)
        outs.append(o.astype(np.float64))
    gL = S["gsnap"][L]
    gidx_of = S["gidx_of"]
    hidvec = {}
    for t in np.unique(t_index):
        t = int(t)
        if US5[t]:
            c = t // NPC0
            col = S["percore"][c]["colmap"][L - 1][t]
            hidvec[t] = outs[c][:, col]
        else:
            gi = gidx_of[t]
            assert gi >= 0
            hidvec[t] = gL[gi].reshape(NQ * D)

    mlp_W1, mlp_b1, mlp_W2, mlp_b2 = W["mlp"]
    query = S["query"]
    Kk = t_index.shape[1]
    score = np.zeros((B, Kk), dtype=np.float32)
    for q in range(B):
        feat = np.stack([
            np.concatenate([hidvec[int(t)][q * D:(q + 1) * D], query[q]])
            for t in t_index[q]])
        hdd = np.maximum(feat @ mlp_W1 + mlp_b1, 0)
        score[q] = ((hdd @ mlp_W2 + mlp_b2)[:, 0]).astype(np.float32)
    return score


# revision 7
# speedup vs baseline: 8.1060x; 1.2106x over previous
"""Trainium2 Bass kernel v4 for NBFNet: exact backward-dependency-cone
truncation (score only needs hidden[t_idx]; restrict each layer to
FW_{l+1} & N_{l+1} nodes), host-side generic evolution for untouched
nodes, compact per-layer AllGather tables, single-strip DVE trees."""
import os
import sys
import types
import numpy as np

N = 50000
R = 100
D = 32
L = 6
B = 4
NQ = 4
EPS = 1e-6
NCORES = 8
NPC0 = N // NCORES
NSEEDN = 4
ZERO_ENTRY = 2 * R          # rel palette: [0,2R) rels, 2R zero, 2R+1.. bnd
ZERO_ROW = 0
ONES_ROW = 1
PAT_BASE = 2
SROWS = 16
SLABTAIL = 1536
MERGE_SLOTS = 96
BUCKETS = np.array([1, 2, 3, 4, 6, 8, 10, 12, 16, 20, 24, 32, 48, 64, 96,
                    128, 192, 256, 384, 512], dtype=np.int64)


def _env_setup():
    if "/opt/trn_rl_repo" not in sys.path:
        sys.path.insert(0, "/opt/trn_rl_repo")
    try:
        import antenv  # noqa
        if "antenv.axon_hooks" not in sys.modules:
            hook = [None]
            mod = types.ModuleType("antenv.axon_hooks")
            mod.set_axon_ntff_profile_hook = lambda h: hook.__setitem__(0, h)
            mod.get_axon_ntff_profile_hook = lambda: hook[0]
            sys.modules["antenv.axon_hooks"] = mod
            antenv.axon_hooks = mod
            try:
                sys.path.insert(0, "/root/.axon_site/trn_agent_boot")
                import trn_boot
                mod.set_axon_ntff_profile_hook(
                    trn_boot._ntff_profile_via_ctypes("/opt/axon/libaxon_pjrt.so"))
            except Exception:
                pass
    except Exception:
        pass


_env_setup()

import ml_dtypes  # noqa: E402

bf16 = ml_dtypes.bfloat16


def _bucket(x):
    return BUCKETS[np.searchsorted(BUCKETS, x)]


def _rup(x, m):
    return (int(x) + m - 1) // m * m


def _wrap_idx(v):
    n = len(v)
    assert n % 16 == 0
    a = np.asarray(v, dtype=np.int16).reshape(n // 16, 16).T
    return np.tile(a, (8, 1))


def build_host(inputs):
    el = np.asarray(inputs["edge_list"])
    src = el[:, 0].astype(np.int64)
    dst = el[:, 1].astype(np.int64)
    rel = el[:, 2].astype(np.int64)
    h_index = np.asarray(inputs["h_index"])
    r_index = np.asarray(inputs["r_index"])
    t_index = np.asarray(inputs["t_index"])
    query_emb = np.asarray(inputs["query_emb"], np.float64)
    lin_W = np.asarray(inputs["lin_W"], np.float64)
    lin_b = np.asarray(inputs["lin_b"], np.float64)
    h0 = h_index[:, 0].astype(np.int64)
    r0 = r_index[:, 0].astype(np.int64)
    query = query_emb[r0]

    # forward wavefront (value-based, per-query then union, as reference)
    T = np.zeros((B, N), dtype=bool)
    T[np.arange(B), h0] = True
    FW = []
    for l in range(L + 1):
        FW.append(T.any(0).copy())
        if l < L:
            for q in range(B):
                T[q, dst[T[q, src]]] = True

    # backward needed sets
    tgts = np.unique(t_index)
    Nl = [None] * (L + 1)
    m = np.zeros(N, dtype=bool)
    m[tgts] = True
    Nl[L] = m
    for l in range(L - 1, -1, -1):
        m2 = Nl[l + 1].copy()
        m2[src[Nl[l + 1][dst]]] = True
        Nl[l] = m2
    US = [FW[l + 1] & Nl[l + 1] for l in range(L)]

    # per-node constants (exact, host fp64)
    indeg = np.bincount(dst, minlength=N)
    degree = indeg.astype(np.float64) + 1.0
    scale = np.log(degree)
    scale = scale / scale.mean()
    iscale = 1.0 / np.clip(scale, 1e-2, None)
    rcnt = 1.0 / degree

    # seeds + boundary patterns
    seeds = np.unique(h0)
    pat = {}
    for n in seeds:
        p = np.zeros(NQ * D)
        for q in range(B):
            if h0[q] == n:
                p[q * D:(q + 1) * D] += query[q]
        pat[int(n)] = p
    is_seed = np.zeros(N, dtype=bool)
    is_seed[seeds] = True
    bnd_entry_of = {int(n): ZERO_ENTRY + 1 + j for j, n in enumerate(seeds)}
    pat_row_of = {int(n): PAT_BASE + j for j, n in enumerate(seeds)}
    NRELE = ZERO_ENTRY + 1 + len(seeds)

    # generic (untouched-node) evolution, host fp64, only for needed nodes
    need_g = np.zeros(N, dtype=bool)
    for l in range(1, L):
        need_g |= US[l] & ~US[l - 1]
    need_g |= Nl[L] & ~US[L - 1]
    need_g &= ~is_seed
    gnodes = np.nonzero(need_g)[0]
    gidx_of = np.full(N, -1, dtype=np.int64)
    gidx_of[gnodes] = np.arange(len(gnodes))
    gsnap = [np.zeros((len(gnodes), B, D))]
    gcur = gsnap[0]
    seps = np.sqrt(EPS)
    for l in range(L):
        Wh = lin_W[l][:D]
        Wu = lin_W[l][D:].reshape(D, 4, 3, D)
        Av = Wu[:, 3, 0, :].sum(0)
        Bv = Wu[:, 3, 1, :].sum(0)
        Cv = Wu[:, 3, 2, :].sum(0)
        std_term = seps * (Av[None, :] + scale[gnodes, None] * Bv[None, :]
                           + iscale[gnodes, None] * Cv[None, :])
        gcur = np.maximum(
            gcur @ Wh + std_term[:, None, :] + lin_b[l][None, None, :], 0.0)
        gsnap.append(gcur)

    # pre-pass: gext sizes (per-core new non-seed cols for layers >= 1)
    core_of = lambda n: n // NPC0  # noqa: E731
    gext_count = np.zeros(NCORES, dtype=np.int64)
    for l in range(1, L):
        new = US[l] & ~US[l - 1] & ~is_seed
        for n in np.nonzero(new)[0]:
            gext_count[core_of(n)] += 1
    GE = _rup(gext_count.max(), 16) if gext_count.max() else 16
    ag0 = SROWS

    e_by_dst = np.argsort(dst, kind="stable")
    dst_s = dst[e_by_dst]

    prev_col = np.full(N, -1, dtype=np.int64)
    prev_Cp = 0
    agoff = [0] * (L + 1)   # agoff[l] = table offset of region used at layer l
    nxt = ag0

    layers = []
    percore = [dict(idxh=[], relc=[], idxp=[], helpers=[], corr=[],
                    gext_rows=[], colmap=[]) for _ in range(NCORES)]

    for l in range(L):
        e_act = FW[l][src] & US[l][dst]
        k_glob = np.bincount(dst[e_act], minlength=N)

        core_stat = []
        for c in range(NCORES):
            n0 = c * NPC0
            rng = np.arange(n0, n0 + NPC0)
            usn = rng[US[l][rng]]
            sd = usn[is_seed[usn]]
            core_stat.append(dict(n0=n0, usn=usn, sd=sd))

        wseed = 2
        for st in core_stat:
            if len(st["sd"]):
                wseed = max(wseed, int(_bucket(k_glob[st["sd"]].max() + 1)))

        ncl = len(BUCKETS)
        Cmat = np.zeros((NCORES, ncl), dtype=np.int64)
        Zv = np.zeros(NCORES, dtype=np.int64)
        for c, st in enumerate(core_stat):
            nonseed = st["usn"][~is_seed[st["usn"]]]
            act = nonseed[k_glob[nonseed] > 0]
            st["act"] = act
            st["zk"] = nonseed[k_glob[nonseed] == 0]
            st["wb"] = np.searchsorted(BUCKETS, k_glob[act])
            Cmat[c] = np.bincount(st["wb"], minlength=ncl)
            Zv[c] = len(st["zk"])

        suf = Cmat[:, ::-1].cumsum(1)[:, ::-1]
        M = suf.max(0)
        counts = M - np.append(M[1:], 0)
        # merge small classes upward (fewer DVE tree ops; bounded slot pad)
        while True:
            nz = [i for i in range(ncl) if counts[i] > 0]
            if len(nz) < 2:
                break
            best = None
            for a, b2 in zip(nz, nz[1:]):
                cost = counts[a] * (BUCKETS[b2] - BUCKETS[a])
                if best is None or cost < best[0]:
                    best = (cost, a, b2)
            if best[0] > MERGE_SLOTS:
                break
            counts[best[2]] += counts[best[1]]
            counts[best[1]] = 0
        Z = int(Zv.max())
        cls = [(int(BUCKETS[i]), int(counts[i])) for i in range(ncl)
               if counts[i] > 0]
        A = NSEEDN + int(counts.sum())
        C = A + Z
        Cp = _rup(max(C, 1), 128)
        assert Cp <= SLABTAIL

        w_seq = np.zeros(A, dtype=np.int64)
        w_seq[:NSEEDN] = wseed
        o = NSEEDN
        for w, nn_ in cls:
            w_seq[o:o + nn_] = w
            o += nn_
        off = np.zeros(A + 1, dtype=np.int64)
        np.cumsum(w_seq, out=off[1:])
        S = int(off[-1])
        Xp = _rup(max(S, 16), 128)

        runs = [(0, 0, NSEEDN, wseed)]
        soff, noff = NSEEDN * wseed, NSEEDN
        for w, nn_ in cls:
            runs.append((soff, noff, nn_, w))
            soff += nn_ * w
            noff += nn_

        layers.append(dict(wseed=wseed, cls=cls, A=A, C=C, Cp=Cp, S=S,
                           Xp=Xp, runs=runs, agoff=agoff[l]))

        new_col = np.full(N, -1, dtype=np.int64)
        for c, st in enumerate(core_stat):
            n0 = c * NPC0
            # non-US nodes of this core usable as dummies
            rngl = np.arange(n0, n0 + NPC0)
            dummies = rngl[~US[l][rngl]][::-1]
            di = 0

            region = list(st["sd"])
            while len(region) < NSEEDN:
                region.append(int(dummies[di]))
                di += 1

            order_desc = np.argsort(-st["wb"], kind="stable")
            plist = st["act"][order_desc]
            pbuck = st["wb"][order_desc]
            placed = {i: [] for i in range(ncl)}
            ptr = 0
            for i in range(ncl - 1, -1, -1):
                cnt = int(counts[i])
                while cnt > 0 and ptr < len(plist):
                    assert pbuck[ptr] <= i
                    placed[i].append(int(plist[ptr]))
                    ptr += 1
                    cnt -= 1
            assert ptr == len(plist)
            node_order = list(region)
            for i in range(ncl):
                needi = int(counts[i]) - len(placed[i])
                fill = [int(dummies[di + j]) for j in range(needi)]
                di += needi
                node_order.extend(placed[i] + fill)
            zlist = list(st["zk"])
            while len(zlist) < Z:
                zlist.append(int(dummies[di]))
                di += 1
            node_order.extend(zlist)
            while len(node_order) < Cp:
                node_order.append(int(dummies[di]))
                di += 1
            node_order = np.array(node_order, dtype=np.int64)
            assert len(node_order) == Cp

            # slot stream
            idxh = np.full(Xp, ONES_ROW, dtype=np.int64)
            relc = np.full(Xp, ZERO_ENTRY, dtype=np.int64)
            rank_of = np.full(N, -1, dtype=np.int64)
            rank_of[node_order[:A]] = np.arange(A)
            lo = np.searchsorted(dst_s, n0)
            hi = np.searchsorted(dst_s, n0 + NPC0)
            ee = e_by_dst[lo:hi]
            ee = ee[e_act[ee]]
            if len(ee):
                rk = rank_of[dst[ee]]
                assert (rk >= 0).all()
                o2 = np.argsort(rk, kind="stable")
                ee = ee[o2]
                rks = rk[o2]
                grp = np.searchsorted(rks, np.arange(A))
                within = np.arange(len(ee)) - grp[rks]
                assert (within < w_seq[rks]).all()
                slotpos = off[rks] + within
                if l == 0:
                    spos = np.array([pat_row_of[int(s)] for s in src[ee]],
                                    dtype=np.int64)
                else:
                    pc = prev_col[src[ee]]
                    assert (pc >= 0).all()
                    spos = agoff[l] + (src[ee] // NPC0) * prev_Cp + pc
                idxh[slotpos] = spos
                relc[slotpos] = rel[ee]

            corr = np.zeros((2, 128, NSEEDN), dtype=np.float64)
            for irank in range(NSEEDN):
                n_ = int(node_order[irank])
                if not (is_seed[n_] and US[l][n_]):
                    continue
                p = pat[n_]
                kk = int(k_glob[n_])
                pad0 = int(off[irank]) + kk
                pad1 = int(off[irank + 1])
                idxh[pad0:pad1] = ONES_ROW
                relc[pad0:pad1] = bnd_entry_of[n_]
                npads = wseed - kk
                assert npads >= 1
                corr[0, :, irank] = (npads - 1) * p
                corr[1, :, irank] = (npads - 1) * p * p

            # hp indices
            idxp = np.full(Cp, SLABTAIL + ONES_ROW, dtype=np.int64)
            for j, n_ in enumerate(node_order):
                n_ = int(n_)
                if not US[l][n_]:
                    continue
                if l == 0:
                    idxp[j] = SLABTAIL + (pat_row_of[n_] if is_seed[n_]
                                          else ZERO_ROW)
                elif prev_col[n_] >= 0:
                    idxp[j] = prev_col[n_]
                else:
                    assert not is_seed[n_]
                    gi = gidx_of[n_]
                    assert gi >= 0
                    row = SROWS + len(percore[c]["gext_rows"])
                    percore[c]["gext_rows"].append(
                        gsnap[l][gi].reshape(NQ * D))
                    idxp[j] = SLABTAIL + row

            hv = np.ones((4, Cp), dtype=np.float64)
            usm = US[l][node_order]
            hv[0, usm] = rcnt[node_order[usm]]
            hv[1, usm] = rcnt[node_order[usm]]
            hv[2, usm] = scale[node_order[usm]]
            hv[3, usm] = iscale[node_order[usm]]

            new_col[node_order[usm]] = np.nonzero(usm)[0]

            percore[c]["idxh"].append(idxh)
            percore[c]["relc"].append(relc)
            percore[c]["idxp"].append(idxp)
            percore[c]["helpers"].append(hv)
            percore[c]["corr"].append(corr)
            percore[c]["colmap"].append(
                dict((int(n2), int(j2)) for j2, n2 in enumerate(node_order)
                     if US[l][n2]))

        prev_col = new_col
        prev_Cp = Cp
        if l < L - 1:
            agoff[l + 1] = nxt
            nxt += NCORES * Cp
    TROWS = nxt
    assert TROWS <= 32767, TROWS

    return dict(layers=layers, percore=percore, query=query, seeds=seeds,
                pat=pat, NRELE=NRELE, bnd_entry_of=bnd_entry_of,
                GE=GE, TROWS=TROWS, t_index=t_index, US=US,
                gsnap=gsnap, gidx_of=gidx_of, is_seed=is_seed)


def build_weights(inputs, S):
    rel_W = np.asarray(inputs["rel_W"], np.float64)
    rel_b = np.asarray(inputs["rel_b"], np.float64)
    lin_W = np.asarray(inputs["lin_W"], np.float64)
    lin_b = np.asarray(inputs["lin_b"], np.float64)
    query = S["query"]
    NRELE = S["NRELE"]

    relpal = np.zeros((L, NRELE, 128), dtype=np.float32)
    for l in range(L):
        remb = (query @ rel_W[l] + rel_b[l]).reshape(B, 2 * R, D)
        relpal[l, :2 * R] = remb.transpose(1, 0, 2).reshape(2 * R, NQ * D)
        for n_old, eid in S["bnd_entry_of"].items():
            relpal[l, eid] = S["pat"][n_old]

    wbd = np.zeros((L, 13, 128, 128), dtype=bf16)
    bias = np.zeros((L, 128, 1), dtype=np.float32)
    for l in range(L):
        Wh = lin_W[l][:D]
        Wu = lin_W[l][D:].reshape(D, 4, 3, D)
        for g in range(4):
            for ks in range(3):
                blk = Wu[:, g, ks, :]
                m = np.zeros((128, 128))
                for q in range(NQ):
                    m[q * D:(q + 1) * D, q * D:(q + 1) * D] = blk
                wbd[l, g * 3 + ks] = m.astype(bf16)
        m = np.zeros((128, 128))
        for q in range(NQ):
            m[q * D:(q + 1) * D, q * D:(q + 1) * D] = Wh
        wbd[l, 12] = m.astype(bf16)
        for q in range(NQ):
            bias[l, q * D:(q + 1) * D, 0] = lin_b[l]

    return dict(relpal=relpal, wbd=wbd, bias=bias,
                mlp=(np.asarray(inputs["mlp_W1"], np.float64),
                     np.asarray(inputs["mlp_b1"], np.float64),
                     np.asarray(inputs["mlp_W2"], np.float64),
                     np.asarray(inputs["mlp_b2"], np.float64)))


def build_program(S):
    import concourse.tile as tile
    from concourse import bacc, mybir
    import contextlib

    layers = S["layers"]
    XT = sum(sl["Xp"] for sl in layers)
    CT = sum(sl["Cp"] for sl in layers)
    Cpmax = max(sl["Cp"] for sl in layers)
    Xpmax = max(sl["Xp"] for sl in layers)
    TROWS = S["TROWS"]
    GE = S["GE"]
    CpL = layers[L - 1]["Cp"]
    SR = SROWS + GE

    nc = bacc.Bacc("TRN2", target_bir_lowering=False, debug=False,
                   num_devices=NCORES)
    dtb = mybir.dt.bfloat16
    dtf = mybir.dt.float32
    dti = mybir.dt.int16
    OP = mybir.AluOpType
    AF = mybir.ActivationFunctionType

    idxh_d = nc.dram_tensor("idxh", [128, XT // 16], dti,
                            kind="ExternalInput")
    idxp_d = nc.dram_tensor("idxp", [128, CT // 16], dti,
                            kind="ExternalInput")
    rs_d = nc.dram_tensor("rs", [128, XT], dtb, kind="ExternalInput")
    help_d = nc.dram_tensor("helpers", [128, 4 * CT], dtb,
                            kind="ExternalInput")
    corr_d = nc.dram_tensor("corr", [128, L * 2 * NSEEDN], dtb,
                            kind="ExternalInput")
    wbd_d = nc.dram_tensor("wbd", [128, L * 13 * 128], dtb,
                           kind="ExternalInput")
    bias_d = nc.dram_tensor("biasl", [128, L], dtf, kind="ExternalInput")
    tstat_d = nc.dram_tensor("tstat", [SR, 128], dtb, kind="ExternalInput")
    ident_d = nc.dram_tensor("ident", [128, 128], dtb, kind="ExternalInput")
    outh_d = nc.dram_tensor("outh", [128, CpL], dtb, kind="ExternalOutput")

    tbl = nc.dram_tensor("tblhbm", [TROWS, 128], dtb, addr_space="Shared")
    slabs = [nc.dram_tensor(f"slab{i}", [SLABTAIL + SR, 128], dtb)
             for i in range(2)]

    coffs = [0]
    xoffs = [0]
    for sl in layers:
        coffs.append(coffs[-1] + sl["Cp"])
        xoffs.append(xoffs[-1] + sl["Xp"])

    with tile.TileContext(nc) as tc:
        ctx = contextlib.ExitStack()
        with ctx, nc.allow_low_precision(reason="bf16 stats by design"):
            pw = ctx.enter_context(tc.tile_pool(name="pw", bufs=1))
            pgq = ctx.enter_context(tc.tile_pool(name="pgq", bufs=2))
            pm = ctx.enter_context(tc.tile_pool(name="pm", bufs=2))
            plvl = ctx.enter_context(tc.tile_pool(name="plvl", bufs=2))
            pgrid = ctx.enter_context(tc.tile_pool(name="pgrid", bufs=2))
            phid = ctx.enter_context(tc.tile_pool(name="phid", bufs=2))
            phn = ctx.enter_context(tc.tile_pool(name="phn", bufs=2))
            pt = ctx.enter_context(tc.tile_pool(name="pt", bufs=2))
            pstg = ctx.enter_context(tc.tile_pool(name="pstg", bufs=2))
            ppsum = ctx.enter_context(tc.tile_pool(name="ppsum", bufs=2,
                                                   space="PSUM"))
            ppsT = ctx.enter_context(tc.tile_pool(name="ppsT", bufs=2,
                                                  space="PSUM"))

            nc.sync.dma_start(out=tbl[0:SROWS], in_=tstat_d[0:SROWS])
            for sb in slabs:
                nc.sync.dma_start(out=sb[SLABTAIL:SLABTAIL + SR],
                                  in_=tstat_d[:])
            ident = pw.tile([128, 128], dtb, tag="ident")
            nc.sync.dma_start(out=ident[:], in_=ident_d[:])
            wbdx = pw.tile([128, L * 13, 128], dtb, tag="wbdx")
            nc.sync.dma_start(
                out=wbdx[:],
                in_=wbd_d[:].rearrange("p (k f) -> p k f", f=128))
            biasx = pw.tile([128, L], dtf, tag="biasx")
            nc.sync.dma_start(out=biasx[:], in_=bias_d[:])
            corx = pw.tile([128, L * 2 * NSEEDN], dtb, tag="corx")
            nc.sync.dma_start(out=corx[:], in_=corr_d[:])
            ihx = pw.tile([128, XT // 16], dti, tag="ihx")
            ipx = pw.tile([128, CT // 16], dti, tag="ipx")
            rsx = pw.tile([128, XT], dtb, tag="rsx")
            hlpx = pw.tile([128, 4 * CT], dtb, tag="hlpx")
            for l in range(L):
                x0, x1 = xoffs[l], xoffs[l + 1]
                c0, c1 = coffs[l], coffs[l + 1]
                nc.sync.dma_start(out=ihx[:, x0 // 16:x1 // 16],
                                  in_=idxh_d[:, x0 // 16:x1 // 16])
                nc.sync.dma_start(out=ipx[:, c0 // 16:c1 // 16],
                                  in_=idxp_d[:, c0 // 16:c1 // 16])
                nc.sync.dma_start(out=rsx[:, x0:x1], in_=rs_d[:, x0:x1])
                nc.sync.dma_start(out=hlpx[:, 4 * c0:4 * c1],
                                  in_=help_d[:, 4 * c0:4 * c1])

            def hp_gather(l):
                Cp_l = layers[l]["Cp"]
                c0 = coffs[l]
                hpb = phid.tile([128, Cpmax], dtb, tag="hp", name="hpb")
                nc.gpsimd.dma_gather(
                    out_ap=hpb[:, :Cp_l].rearrange("p (c n) -> p c n", c=1),
                    in_ap=slabs[(l + 1) % 2][:],
                    idxs_ap=ipx[:, c0 // 16:(c0 + Cp_l) // 16],
                    num_idxs=Cp_l, num_idxs_reg=Cp_l,
                    elem_size=128, transpose=True, single_packet=False)
                return hpb[:, :Cp_l]

            hp_next = hp_gather(0)

            for l in range(L):
                sl = layers[l]
                Cp, Xp, A = sl["Cp"], sl["Xp"], sl["A"]
                xoff, coff = xoffs[l], coffs[l]
                wbd = wbdx[:, l * 13:(l + 1) * 13, :]
                biasv = biasx[:, l:l + 1]
                corrt = corx[:, l * 2 * NSEEDN:(l + 1) * 2 * NSEEDN]\
                    .rearrange("p (k f) -> p k f", k=2)
                hlp = hlpx[:, 4 * coff:4 * coff + 4 * Cp]\
                    .rearrange("p (k f) -> p k f", k=4)
                hp = hp_next

                gq = pgq.tile([128, 2, Xpmax], dtb, tag="gq", name="gq")
                nc.gpsimd.dma_gather(
                    out_ap=gq[:, 0:1, :Xp],
                    in_ap=tbl[:],
                    idxs_ap=ihx[:, xoff // 16:(xoff + Xp) // 16],
                    num_idxs=Xp, num_idxs_reg=Xp,
                    elem_size=128, transpose=True, single_packet=False)
                nc.vector.tensor_tensor(out=gq[:, 0, :Xp],
                                        in0=gq[:, 0, :Xp],
                                        in1=rsx[:, xoff:xoff + Xp],
                                        op=OP.mult)
                nc.scalar.activation(gq[:, 1, :Xp], gq[:, 0, :Xp],
                                     AF.Square)

                # grid planes: 0=sum 1=sq 2=max 3=min
                grid3 = pgrid.tile([128, 4, Cpmax], dtb, tag="grid",
                                   name="grid3")
                if A < Cp:
                    nc.vector.memset(grid3[:, :, A:Cp], 0.0)
                for (toff, tnode, nseg, w) in sl["runs"]:
                    _tree(nc, plvl, OP.add, gq, 0, 2, toff, nseg, w,
                          grid3, 0, tnode, mybir, Xpmax)
                    _tree(nc, plvl, OP.max, gq, 0, 1, toff, nseg, w,
                          grid3, 2, tnode, mybir, Xpmax)
                    _tree(nc, plvl, OP.min, gq, 0, 1, toff, nseg, w,
                          grid3, 3, tnode, mybir, Xpmax)

                nc.vector.tensor_tensor(out=grid3[:, 0:2, :NSEEDN],
                                        in0=grid3[:, 0:2, :NSEEDN],
                                        in1=corrt[:], op=OP.subtract)
                nc.vector.tensor_scalar_max(grid3[:, 2, NSEEDN:Cp],
                                            grid3[:, 2, NSEEDN:Cp], 0.0)
                nc.vector.tensor_scalar_min(grid3[:, 3, NSEEDN:Cp],
                                            grid3[:, 3, NSEEDN:Cp], 0.0)
                nc.vector.tensor_tensor(out=grid3[:, 0:2, :Cp],
                                        in0=grid3[:, 0:2, :Cp],
                                        in1=hlp[:, 0:2, :Cp], op=OP.mult)
                msc = pm.tile([128, Cpmax], dtb, tag="msc",
                              name="msc")[:, :Cp]
                nc.scalar.activation(msc[:], grid3[:, 0, :Cp], AF.Square)
                nc.vector.tensor_tensor(out=grid3[:, 1, :Cp],
                                        in0=grid3[:, 1, :Cp],
                                        in1=msc[:], op=OP.subtract)
                nc.vector.tensor_scalar_max(grid3[:, 1, :Cp],
                                            grid3[:, 1, :Cp], EPS)
                nc.scalar.activation(grid3[:, 1, :Cp], grid3[:, 1, :Cp],
                                     AF.Sqrt)

                hnew = phn.tile([128, Cpmax], dtb, tag="hn",
                                name="hnew")[:, :Cp]
                gl = [grid3[:, 0], grid3[:, 2], grid3[:, 3], grid3[:, 1]]
                mt0 = 0
                while mt0 < Cp:
                    mt = min(512, Cp - mt0)
                    ps = [ppsum.tile([128, 512], dtf, tag=f"ps{k2}",
                                     name=f"ps{k2}") for k2 in range(3)]
                    for ks in range(3):
                        for g in range(4):
                            nc.tensor.matmul(
                                ps[ks][:, :mt], wbd[:, g * 3 + ks, :],
                                gl[g][:, mt0:mt0 + mt],
                                start=(g == 0),
                                stop=(g == 3 and ks != 0))
                    nc.tensor.matmul(ps[0][:, :mt], wbd[:, 12, :],
                                     hp[:, mt0:mt0 + mt],
                                     start=False, stop=True)
                    t1 = pt.tile([128, 512], dtb, tag="t1")
                    nc.vector.tensor_tensor(out=t1[:, :mt],
                                            in0=ps[1][:, :mt],
                                            in1=hlp[:, 2, mt0:mt0 + mt],
                                            op=OP.mult)
                    t2 = pt.tile([128, 512], dtb, tag="t2")
                    nc.vector.tensor_tensor(out=t2[:, :mt],
                                            in0=ps[2][:, :mt],
                                            in1=hlp[:, 3, mt0:mt0 + mt],
                                            op=OP.mult)
                    nc.vector.tensor_tensor(out=t1[:, :mt], in0=t1[:, :mt],
                                            in1=t2[:, :mt], op=OP.add)
                    nc.vector.tensor_tensor(out=t1[:, :mt], in0=t1[:, :mt],
                                            in1=ps[0][:, :mt], op=OP.add)
                    nc.scalar.activation(hnew[:, mt0:mt0 + mt], t1[:, :mt],
                                         AF.Relu, bias=biasv)
                    mt0 += mt

                if l == L - 1:
                    nc.sync.dma_start(out=outh_d[:], in_=hnew[:])
                else:
                    slab = slabs[l % 2]
                    for rk in range(Cp // 128):
                        psT = ppsT.tile([128, 128], dtb, tag="psT")
                        nc.tensor.transpose(
                            psT[:], hnew[:, rk * 128:(rk + 1) * 128],
                            ident[:])
                        stg = pstg.tile([128, 128], dtb, tag="stg")
                        nc.scalar.activation(stg[:], psT[:], AF.Copy)
                        nc.sync.dma_start(
                            out=slab[rk * 128:(rk + 1) * 128, :], in_=stg[:])
                    hp_next = hp_gather(l + 1)
                    nc.gpsimd.collective_compute(
                        "AllGather", OP.bypass,
                        replica_groups=[list(range(NCORES))],
                        ins=[slab[0:Cp]],
                        outs=[tbl[layers[l + 1]["agoff"]:
                                  layers[l + 1]["agoff"] + NCORES * Cp]
                              .rearrange("(c n) d -> c n d", c=NCORES)])

    nc.compile()
    return nc


def _tree(nc, plvl, op, src3, p0, np_, toff, nseg, w, grid3, g0, noff,
          mybir, lvlw):
    dtb = mybir.dt.bfloat16
    gout = grid3[:, g0:g0 + np_, noff:noff + nseg]
    if w == 1:
        nc.vector.tensor_copy(gout, src3[:, p0:p0 + np_, toff:toff + nseg])
        return
    cur, cp0, cof, m = src3, p0, toff, w
    while m > 1:
        h = m // 2
        odd = m - 2 * h
        last = (h == 1 and odd == 0)
        if last:
            nxt, nof = grid3[:, g0:g0 + np_], noff
        else:
            nxt = plvl.tile([128, 2, lvlw // 2], dtb, tag="lvl",
                            name="lvl")[:, :np_]
            nof = 0
        nc.vector.tensor_tensor(
            out=nxt[:, :, nof:nof + h * nseg],
            in0=cur[:, cp0:cp0 + np_, cof:cof + h * nseg],
            in1=cur[:, cp0:cp0 + np_, cof + h * nseg:cof + 2 * h * nseg],
            op=op)
        if odd:
            if h == 1:
                nc.vector.tensor_tensor(
                    out=gout,
                    in0=nxt[:, :, nof:nof + nseg],
                    in1=cur[:, cp0:cp0 + np_,
                            cof + 2 * h * nseg:cof + (2 * h + 1) * nseg],
                    op=op)
                return
            nc.vector.tensor_tensor(
                out=nxt[:, :, nof:nof + nseg],
                in0=nxt[:, :, nof:nof + nseg],
                in1=cur[:, cp0:cp0 + np_,
                        cof + 2 * h * nseg:cof + (2 * h + 1) * nseg],
                op=op)
        cur, cp0, cof, m = nxt, 0, nof, h


_RUN_STATE = {}


def kernel(**inputs):
    from concourse.bass_utils import run_bass_kernel_spmd

    S = build_host(inputs)
    W = build_weights(inputs, S)
    nc = build_program(S)

    layers = S["layers"]
    XT = sum(sl["Xp"] for sl in layers)
    CT = sum(sl["Cp"] for sl in layers)
    GE = S["GE"]

    # static table rows (shared): zero, ones, patterns
    tstat_base = np.zeros((SROWS + GE, 128), dtype=bf16)
    tstat_base[ONES_ROW] = bf16(1.0)
    for n_old, p in S["pat"].items():
        tstat_base[PAT_BASE + (S["bnd_entry_of"][n_old] - ZERO_ENTRY - 1)] \
            = p.astype(bf16)

    in_maps = []
    for c in range(NCORES):
        pc = S["percore"][c]
        ihw = _wrap_idx(np.concatenate(pc["idxh"]))
        ipw = _wrap_idx(np.concatenate(pc["idxp"]))
        relc = np.concatenate(pc["relc"])
        rsb = np.zeros((128, XT), dtype=bf16)
        xo = 0
        for l in range(L):
            nl = len(pc["relc"][l])
            rsb[:, xo:xo + nl] = \
                W["relpal"][l][relc[xo:xo + nl]].T.astype(bf16)
            xo += nl
        hlpflat = np.concatenate(
            [pc["helpers"][l].reshape(-1) for l in range(L)])
        hlpb = np.broadcast_to(hlpflat[None, :].astype(bf16),
                               (128, 4 * CT)).copy()
        corrflat = np.concatenate(
            [pc["corr"][l].transpose(1, 0, 2).reshape(128, 2 * NSEEDN)
             for l in range(L)], axis=1).astype(bf16)
        tstat = tstat_base.copy()
        assert len(pc["gext_rows"]) <= GE
        for i, row in enumerate(pc["gext_rows"]):
            tstat[SROWS + i] = row.astype(bf16)
        in_maps.append(dict(
            idxh=ihw, idxp=ipw, rs=rsb, helpers=hlpb, corr=corrflat,
            wbd=np.ascontiguousarray(
                W["wbd"].transpose(2, 0, 1, 3).reshape(128, -1)),
            biasl=np.ascontiguousarray(W["bias"][:, :, 0].T),
            tstat=tstat, ident=np.eye(128, dtype=bf16)))

    res = run_bass_kernel_spmd(nc, in_maps, core_ids=list(range(NCORES)),
                               trace=bool(os.environ.get("NBF_TRACE")))
    _RUN_STATE["exec_time_ns"] = res.exec_time_ns

    t_index = S["t_index"]
    US5 = S["US"][L - 1]
    outs = []
    for c in range(NCORES):
        o = np.asarray(res.results[c]["outh"])
        if o.dtype != bf16:
            o = o.view(bf16)
        outs.append(o.astype(np.float64))
    gL = S["gsnap"][L]
    gidx_of = S["gidx_of"]
    hidvec = {}
    for t in np.unique(t_index):
        t = int(t)
        if US5[t]:
            c = t // NPC0
            col = S["percore"][c]["colmap"][L - 1][t]
            hidvec[t] = outs[c][:, col]
        else:
            gi = gidx_of[t]
            assert gi >= 0
            hidvec[t] = gL[gi].reshape(NQ * D)

    mlp_W1, mlp_b1, mlp_W2, mlp_b2 = W["mlp"]
    query = S["query"]
    Kk = t_index.shape[1]
    score = np.zeros((B, Kk), dtype=np.float32)
    for q in range(B):
        feat = np.stack([
            np.concatenate([hidvec[int(t)][q * D:(q + 1) * D], query[q]])
            for t in t_index[q]])
        hdd = np.maximum(feat @ mlp_W1 + mlp_b1, 0)
        score[q] = ((hdd @ mlp_W2 + mlp_b2)[:, 0]).astype(np.float32)
    return score
